# revision 1
# baseline (speedup 1.0000x reference)
"""Trainium2 Bass kernel for DynamicPathCrossAttention.

Sharding: batch-parallel — core b computes batch element b end-to-end. The
path-gating MLP is evaluated on the host from the runtime inputs; each core
only computes cross-attention for its batch element's TOP_K=2 selected paths.

Weight folding (host, shared across cores): because the reference chain is
linear around the softmax, adjacent projection pairs collapse:
  logits = Q Wq^T Wk S^T          -> G_p = Wq^T @ Wk_p     (logits = Q G S^T)
  out    = attn S Wv^T Wo^T (...) -> H_p = Wo @ Wv_p       (out = attn S H^T)
so the device never materializes Qp, K, or V — 8 big matmul units per core
instead of 10. Bias algebra: the per-q logit terms cancel inside softmax; the
per-k term ships as an exp() bias column vb = (S @ Wk^T bq) / sqrt(D); bv
folds into an effective output bias boe = bo + sum_p w_p (Wo @ bv_p).

Device pipeline per path (all contractions on SBUF partitions, zero
on-device transposes; all matmuls float32r = full PE rate, ~1e-4 rel err):
  TMP[d',q]   = sum_d  G[d,d'] QT[d,q]        (lhsT=G resident, rhs=QT chunk)
  logitsT[k,q]= sum_d' ST[d',k] TMP[d',q]     (lhsT=ST resident, rhs=TMP)
  expT        = exp(logitsT/sqrt(D) + vb[k])  (ACT from PSUM, bias fused)
  rowsum[1,q] = sum_k expT[k,q]               (ones-matmul)
  AOS[d',q]   = sum_k SN[k,d'] expT[k,q]      (lhsT=SN chunk, rhs=expT)
  AOSs        = AOS * (w_p/rowsum broadcast)  (DVE from PSUM)
  outT[o,q]  += sum_d' HT[d',o] AOSs[d',q]    (lhsT=HT resident; path-0 half
                                               stashed in SBUF, path-1 adds)
"""

import numpy as np

D = 1024
P = 4
TOP_K = 2
B = 8
LQ = 1024
LK = 1024
N_CORES = 8

_CACHE = {}


def _build_program():
    import concourse.bass as bass  # noqa: F401
    import concourse.mybir as mybir
    import concourse.tile as tile
    from concourse import bacc

    f32 = mybir.dt.float32
    f32r = mybir.dt.float32r
    Exp = mybir.ActivationFunctionType.Exp
    Identity = mybir.ActivationFunctionType.Identity
    ADD = mybir.AluOpType.add
    MULT = mybir.AluOpType.mult

    nc = bacc.Bacc(
        "TRN2", target_bir_lowering=False, debug=False, enable_asserts=False
    )

    def din(name, shape):
        return nc.dram_tensor(name, shape, f32, kind="ExternalInput").ap()

    QT = din("QT", [D, LQ])
    ST_d = [din(f"S{p}T", [D, LK]) for p in range(2)]
    SN_d = [din(f"SN{p}", [LK, D]) for p in range(2)]
    G_d = [din(f"G{p}", [D, D]) for p in range(2)]
    HT_d = [din(f"HT{p}", [D, D]) for p in range(2)]
    vb_d = [din(f"vb{p}", [LK, 1]) for p in range(2)]
    boe_c = din("boe", [D, 1])
    wgt = din("wgt", [1, 2])
    ones_col_d = din("ones_col", [128, 1])
    ones_row_d = din("ones_row", [1, 128])
    outT = nc.dram_tensor("outT", [D, LQ], f32, kind="ExternalOutput").ap()

    SCALE = 1.0 / float(np.sqrt(D))
    nD = D // 128

    with tile.TileContext(nc) as tc:
        import contextlib

        with contextlib.ExitStack() as ctx:
            const = ctx.enter_context(tc.tile_pool(name="const", bufs=1))
            stream = ctx.enter_context(tc.tile_pool(name="stream", bufs=10))
            tap = ctx.enter_context(tc.tile_pool(name="tap", bufs=1))
            stp = ctx.enter_context(tc.tile_pool(name="stp", bufs=1))
            kvp = ctx.enter_context(tc.tile_pool(name="kvp", bufs=1))
            expp = ctx.enter_context(tc.tile_pool(name="expp", bufs=1))
            o0p = ctx.enter_context(tc.tile_pool(name="o0p", bufs=1))
            smallp = ctx.enter_context(tc.tile_pool(name="smallp", bufs=2))
            vecp = ctx.enter_context(tc.tile_pool(name="vecp", bufs=1))
            osbp = ctx.enter_context(tc.tile_pool(name="osbp", bufs=4))
            psp = ctx.enter_context(tc.tile_pool(name="psp", bufs=8, space="PSUM"))
            dramp = ctx.enter_context(tc.tile_pool(name="dramp", bufs=2, space="DRAM"))

            # ---- constants (DMAs deferred behind the first compute chunks) --
            ones_col = const.tile([128, 1], f32r)
            vb_t = [const.tile([128, nD], f32, name=f"vb_t{p}") for p in range(2)]
            boe_t = const.tile([128, nD], f32)
            wgt_sb = const.tile([1, 2], f32)

            def emit_const_dmas():
                nc.sync.dma_start(ones_col[:], ones_col_d[:].bitcast(f32r))
                for p in range(2):
                    nc.sync.dma_start(
                        vb_t[p][:], vb_d[p].rearrange("(t p) o -> p (t o)", p=128)
                    )
                nc.sync.dma_start(
                    boe_t[:], boe_c.rearrange("(t p) o -> p (t o)", p=128)
                )
                nc.sync.dma_start(wgt_sb[:], wgt[:])

            def load_st_tile(p, d_t):
                s_tile = stp.tile([128, LK], f32r, tag=f"st{d_t}", name=f"st{d_t}")
                nc.sync.dma_start(
                    s_tile[:],
                    ST_d[p][d_t * 128 : (d_t + 1) * 128, :].bitcast(f32r),
                )
                return s_tile

            out0 = [
                o0p.tile([128, LQ], f32, name=f"out0_{i}") for i in range(nD)
            ]

            for p in range(2):
                # =====================================================
                # TMP[d', q] = sum_d G[d, d'] QT[d, q]
                # G resident in kv slots; QT streams once per path.
                # ST for this path trickles in behind.
                # =====================================================
                g_res = []
                st = []
                tmp_t = [
                    tap.tile([128, LQ], f32r, tag=f"ta{i}", name=f"tmp{i}")
                    for i in range(nD)
                ]
                for q_b in range(2):
                    ps_t = [
                        psp.tile([128, 512], f32, tag="acc", name="ps_t")
                        for _ in range(8)
                    ]
                    for d_t in range(8):
                        qt_ch = stream.tile([128, 512], f32r, tag="wc", name="qtc")
                        nc.sync.dma_start(
                            qt_ch[:],
                            QT[
                                d_t * 128 : (d_t + 1) * 128,
                                q_b * 512 : (q_b + 1) * 512,
                            ].bitcast(f32r),
                        )
                        if q_b == 0:
                            if p == 0 and d_t == 0:
                                # two independently-waitable half tiles so the
                                # first matmuls start on the first 256KB
                                ga = kvp.tile([128, 512], f32r, tag="kv0a", name="g0a")
                                nc.sync.dma_start(
                                    ga[:], G_d[p][0:128, 0:512].bitcast(f32r)
                                )
                                gb = kvp.tile([128, 512], f32r, tag="kv0b", name="g0b")
                                nc.sync.dma_start(
                                    gb[:], G_d[p][0:128, 512:1024].bitcast(f32r)
                                )
                                g_res.append((ga, gb))
                            else:
                                g_tile = kvp.tile(
                                    [128, D], f32r, tag=f"kv{d_t}", name=f"g{d_t}"
                                )
                                nc.sync.dma_start(
                                    g_tile[:],
                                    G_d[p][
                                        d_t * 128 : (d_t + 1) * 128, :
                                    ].bitcast(f32r),
                                )
                                g_res.append(g_tile)
                        if p == 0 and q_b == 0 and d_t == 2:
                            emit_const_dmas()
                        for dp_t in range(8):
                            g = g_res[d_t]
                            if isinstance(g, tuple):
                                lhsT = (
                                    g[0][:, dp_t * 128 : (dp_t + 1) * 128]
                                    if dp_t < 4
                                    else g[1][:, (dp_t - 4) * 128 : (dp_t - 3) * 128]
                                )
                            else:
                                lhsT = g[:, dp_t * 128 : (dp_t + 1) * 128]
                            nc.tensor.matmul(
                                ps_t[dp_t][:],
                                lhsT,
                                qt_ch[:],
                                start=(d_t == 0),
                                stop=(d_t == 7),
                            )
                        # trickle this path's S^T behind the TMP chunks
                        if q_b == 1 and d_t in (0, 2, 4, 6):
                            st.append(load_st_tile(p, len(st)))
                    for dp_t in range(8):
                        dst = tmp_t[dp_t][:, q_b * 512 : (q_b + 1) * 512]
                        if dp_t % 2 == 0:
                            nc.scalar.activation(dst, ps_t[dp_t][:], Identity)
                        else:
                            nc.vector.tensor_copy(dst, ps_t[dp_t][:])
                while len(st) < 8:
                    st.append(load_st_tile(p, len(st)))

                # =====================================================
                # logits + exp + row-sums (both q blocks)
                # =====================================================
                expt = [
                    [
                        expp.tile([128, 512], f32r, tag=f"ex{q_b}_{k_t}", name="expt")
                        for k_t in range(8)
                    ]
                    for q_b in range(2)
                ]
                sbc = [None, None]

                def emit_logits_exp(q_b):
                    for k_t in range(8):
                        ps = psp.tile([128, 512], f32, tag="acc", name="ps_l")
                        for dp_t in range(8):
                            nc.tensor.matmul(
                                ps[:],
                                st[dp_t][:, k_t * 128 : (k_t + 1) * 128],
                                tmp_t[dp_t][:, q_b * 512 : (q_b + 1) * 512],
                                start=(dp_t == 0),
                                stop=(dp_t == 7),
                            )
                        nc.scalar.activation(
                            expt[q_b][k_t][:],
                            ps[:],
                            Exp,
                            bias=vb_t[p][:, k_t : k_t + 1],
                            scale=SCALE,
                        )

                def emit_rowsum(q_b):
                    ps_s = psp.tile([1, 512], f32, tag="acc", name="ps_s")
                    for k_t in range(8):
                        nc.tensor.matmul(
                            ps_s[:],
                            ones_col[:],
                            expt[q_b][k_t][:],
                            start=(k_t == 0),
                            stop=(k_t == 7),
                        )
                    return ps_s

                def emit_sbc(q_b, ps_s):
                    rs = vecp.tile([1, 512], f32, tag="rs", name="rs")
                    nc.vector.reciprocal(rs[:], ps_s[:])
                    s_row = vecp.tile([1, 512], f32, tag="srow", name="s_row")
                    nc.vector.tensor_scalar_mul(s_row[:], rs[:], wgt_sb[0:1, p : p + 1])
                    # broadcast across partitions via a DRAM bounce (the PE
                    # stays out of it; DRAM-source partition_broadcast works)
                    srow_d = dramp.tile([1, 512], f32, tag="srd", name="srow_d")
                    nc.sync.dma_start(srow_d[:], s_row[:])
                    sb_t = smallp.tile([128, 512], f32, tag="sbc", name="sb_t")
                    nc.sync.dma_start(sb_t[:], srow_d[0:1, :].partition_broadcast(128))
                    sbc[q_b] = sb_t

                emit_logits_exp(0)
                ps_s0 = emit_rowsum(0)
                emit_logits_exp(1)
                emit_sbc(0, ps_s0)
                ps_s1 = emit_rowsum(1)
                emit_sbc(1, ps_s1)

                # HT resident: reuse the (now dead) ST slots
                ht_res = []
                for dp_t in range(8):
                    h_tile = stp.tile(
                        [128, D], f32r, tag=f"st{dp_t}", name=f"ht{dp_t}"
                    )
                    nc.sync.dma_start(
                        h_tile[:],
                        HT_d[p][dp_t * 128 : (dp_t + 1) * 128, :].bitcast(f32r),
                    )
                    ht_res.append(h_tile)

                # =====================================================
                # AOS[d', q] = sum_k SN[k, d'] expT[k, q], then scale by
                # sbc = w_p / rowsum  (PSUM -> SBUF fused with the copy)
                # =====================================================
                aoss = [
                    tap.tile([128, LQ], f32r, tag=f"ta{i}", name=f"aoss{i}")
                    for i in range(nD)
                ]
                for dp_h in range(2):
                    ps_a = [
                        [
                            psp.tile([128, 512], f32, tag="acc", name="ps_a")
                            for _ in range(2)
                        ]
                        for _ in range(4)
                    ]
                    for k_t in range(8):
                        snc = stream.tile([128, 512], f32r, tag="wc", name="snc")
                        nc.sync.dma_start(
                            snc[:],
                            SN_d[p][
                                k_t * 128 : (k_t + 1) * 128,
                                dp_h * 512 : (dp_h + 1) * 512,
                            ].bitcast(f32r),
                        )
                        for dp_i in range(4):
                            for q_b in range(2):
                                nc.tensor.matmul(
                                    ps_a[dp_i][q_b][:],
                                    snc[:, dp_i * 128 : (dp_i + 1) * 128],
                                    expt[q_b][k_t][:],
                                    start=(k_t == 0),
                                    stop=(k_t == 7),
                                )
                    for dp_i in range(4):
                        dp_t = dp_h * 4 + dp_i
                        for q_b in range(2):
                            nc.vector.tensor_tensor(
                                aoss[dp_t][:, q_b * 512 : (q_b + 1) * 512],
                                ps_a[dp_i][q_b][:],
                                sbc[q_b][:],
                                MULT,
                            )

                # =====================================================
                # outT[o, q] += sum_d' HT[d', o] AOSs[d', q]
                # path 0 stashes into SBUF (with boe bias); path 1 adds
                # and writes out.  o_t-outer so copy+DMA pipelines.
                # =====================================================
                for q_b in range(2):
                    for o_t in range(8):
                        ps = psp.tile([128, 512], f32, tag="acc", name="ps_o")
                        for dp_t in range(8):
                            nc.tensor.matmul(
                                ps[:],
                                ht_res[dp_t][:, o_t * 128 : (o_t + 1) * 128],
                                aoss[dp_t][:, q_b * 512 : (q_b + 1) * 512],
                                start=(dp_t == 0),
                                stop=(dp_t == 7),
                            )
                        if p == 0:
                            dst = out0[o_t][:, q_b * 512 : (q_b + 1) * 512]
                            if o_t % 2 == 0:
                                nc.scalar.activation(
                                    dst, ps[:], Identity,
                                    bias=boe_t[:, o_t : o_t + 1],
                                )
                            else:
                                nc.vector.tensor_scalar_add(
                                    dst, ps[:], boe_t[:, o_t : o_t + 1]
                                )
                        else:
                            osb = osbp.tile([128, 512], f32, tag="osb", name="osb")
                            if o_t == 7 and q_b == 1:
                                # split the final tile so copy and DMA pipeline
                                for h in range(2):
                                    sl = slice(h * 256, (h + 1) * 256)
                                    nc.vector.tensor_tensor(
                                        osb[:, sl],
                                        ps[:, sl],
                                        out0[o_t][:, q_b * 512 + h * 256 : q_b * 512 + (h + 1) * 256],
                                        ADD,
                                    )
                                    nc.sync.dma_start(
                                        outT[
                                            o_t * 128 : (o_t + 1) * 128,
                                            q_b * 512 + h * 256 : q_b * 512 + (h + 1) * 256,
                                        ],
                                        osb[:, sl],
                                    )
                            else:
                                nc.vector.tensor_tensor(
                                    osb[:],
                                    ps[:],
                                    out0[o_t][:, q_b * 512 : (q_b + 1) * 512],
                                    ADD,
                                )
                                nc.sync.dma_start(
                                    outT[
                                        o_t * 128 : (o_t + 1) * 128,
                                        q_b * 512 : (q_b + 1) * 512,
                                    ],
                                    osb[:],
                                )

    nc.compile()
    return nc


def _get_program():
    if "nc" not in _CACHE:
        _CACHE["nc"] = _build_program()
    return _CACHE["nc"]


def _host_gating(Q, Wq, bq, Wm1, bm1, Wm2, bm2):
    """Replicates the reference path-score MLP + top-k sparse weights."""
    Qm = Q.astype(np.float64).mean(axis=1)  # [B, D]
    pooled = Qm @ Wq.astype(np.float64).T + bq.astype(np.float64)
    h = np.maximum(pooled @ Wm1.astype(np.float64).T + bm1.astype(np.float64), 0.0)
    pl = h @ Wm2.astype(np.float64).T + bm2.astype(np.float64)  # [B, P]
    pl = pl - pl.max(axis=1, keepdims=True)
    e = np.exp(pl)
    scores = e / e.sum(axis=1, keepdims=True)
    idx = np.argsort(-scores, axis=1, kind="stable")[:, :TOP_K]  # [B, 2]
    w = np.take_along_axis(scores, idx, axis=1)
    wn = w / (w.sum(axis=1, keepdims=True) + 1e-8)
    return idx.astype(np.int64), wn.astype(np.float32)


def kernel(**inputs):
    from concourse.bass_utils import run_bass_kernel_spmd

    Q = np.asarray(inputs["Q"], dtype=np.float32)
    src = np.asarray(inputs["src"], dtype=np.float32)
    Wq = np.asarray(inputs["Wq"], dtype=np.float32)
    bq = np.asarray(inputs["bq"], dtype=np.float32)
    Wk = np.asarray(inputs["Wk"], dtype=np.float32)
    bk = np.asarray(inputs["bk"], dtype=np.float32)  # noqa: F841  (cancels in softmax)
    Wv = np.asarray(inputs["Wv"], dtype=np.float32)
    bv = np.asarray(inputs["bv"], dtype=np.float32)
    Wm1 = np.asarray(inputs["Wm1"], dtype=np.float32)
    bm1 = np.asarray(inputs["bm1"], dtype=np.float32)
    Wm2 = np.asarray(inputs["Wm2"], dtype=np.float32)
    bm2 = np.asarray(inputs["bm2"], dtype=np.float32)
    Wo = np.asarray(inputs["Wo"], dtype=np.float32)
    bo = np.asarray(inputs["bo"], dtype=np.float32)

    idx, wn = _host_gating(Q, Wq, bq, Wm1, bm1, Wm2, bm2)
    SCALE = 1.0 / float(np.sqrt(D))

    nc = _get_program()

    # host-folded weights, shared across cores (<=4 selected paths)
    sel = sorted(set(idx.flatten().tolist()))
    WqT = Wq.T
    G = {p: np.ascontiguousarray(WqT @ Wk[p]) for p in sel}
    HT = {p: np.ascontiguousarray((Wo @ Wv[p]).T) for p in sel}
    g2 = {p: Wk[p].T @ bq for p in sel}
    Wobv = {p: Wo @ bv[p] for p in sel}
    ones_col = np.ones((128, 1), np.float32)
    ones_row = np.ones((1, 128), np.float32)

    in_maps = []
    for b in range(B):
        p0, p1 = int(idx[b, 0]), int(idx[b, 1])
        boe = bo + wn[b, 0] * Wobv[p0] + wn[b, 1] * Wobv[p1]
        m = {
            "QT": np.ascontiguousarray(Q[b].T),
            "S0T": np.ascontiguousarray(src[p0, b].T),
            "S1T": np.ascontiguousarray(src[p1, b].T),
            "SN0": np.ascontiguousarray(src[p0, b]),
            "SN1": np.ascontiguousarray(src[p1, b]),
            "G0": G[p0],
            "G1": G[p1],
            "HT0": HT[p0],
            "HT1": HT[p1],
            "vb0": np.ascontiguousarray(
                ((src[p0, b] @ g2[p0]) * SCALE).reshape(LK, 1).astype(np.float32)
            ),
            "vb1": np.ascontiguousarray(
                ((src[p1, b] @ g2[p1]) * SCALE).reshape(LK, 1).astype(np.float32)
            ),
            "boe": np.ascontiguousarray(boe.reshape(D, 1).astype(np.float32)),
            "wgt": np.ascontiguousarray(wn[b].reshape(1, 2)),
            "ones_col": ones_col,
            "ones_row": ones_row,
        }
        in_maps.append(m)

    res = run_bass_kernel_spmd(nc, in_maps, core_ids=list(range(N_CORES)))
    out = np.stack([res.results[b]["outT"].T for b in range(B)], axis=0)
    return np.ascontiguousarray(out).astype(np.float32)



# revision 13
# speedup vs baseline: 1.3470x; 1.3470x over previous
"""Trainium2 Bass kernel for DynamicPathCrossAttention.

Sharding: batch-parallel — core b computes batch element b end-to-end. The
path-gating MLP runs on the host; each core computes cross-attention for its
batch element's TOP_K=2 selected paths only.

Weight folding (host, shared across cores): the chain is linear around the
softmax, so adjacent projection pairs collapse:
  logits = Q Wq^T Wk S^T          -> G_p = Wq^T @ Wk_p     (logits = Q G S^T)
  out    = attn S Wv^T Wo^T (...) -> H_p = Wo @ Wv_p       (out = attn S H^T)
Bias algebra: per-q logit terms cancel in softmax; the per-k term ships as an
exp() bias column vb; bv folds into an effective output bias boe.

All matmuls run as fp8e4 (e4m3) DoubleRow with hi/lo error compensation:
every operand X is split on host (or on device for intermediates) into
  X_hi = fp8(X*s),  X_lo = fp8(X*s - X_hi)
and each contraction A@B is computed as A_hi@B_hi + A_lo@B_hi + A_hi@B_lo
(the lo*lo term ~eps^2 is dropped), keeping rel err ~2e-3 at 2x the f32r
matmul rate. DoubleRow packs two 128-row contraction planes per matmul:
operands are laid out [128 part, 2 planes, cols]; logical contraction index
d = 256*j + 128*i + p for pair-tile j, plane i, partition p.

Per-core pipeline (scales are powers of 2, folded into ACT scale factors):
  S1: TMP[d',q] = G^T QT      (G,QT host fp8 pairs; TMP hi/lo via ACT+DVE)
  S2: logitsT[k,q] = ST^T TMP ; expT = exp(.*SCALE + vb + ln sE) (ACT, f32)
      exp hi cast (ACT) + lo residual (Pool tensor_tensor sub)
  rowsum = DR ones-matmul over exp hi+lo pairs; sbc = w/rowsum broadcast
      (per-q) via DRAM-bounce partition_broadcast
  S3: AOS[d',q] = SN^T expT ; AOSs = AOS * sbc (DVE), hi cast (DVE) +
      lo residual (Pool)
  S4 (both paths in one accumulation, contraction 2048):
      outT[o,q] = sum_p HT_p^T AOSs_p ; final ACT copy applies 2^-19 + boe.

A memset-fed warmup matmul chain keeps the PE busy from t~1us so the
p-state ramp completes before the first real matmul (DMA-latency window).
"""

import numpy as np
import ml_dtypes

D = 1024
P = 4
TOP_K = 2
B = 8
LQ = 1024
LK = 1024
N_CORES = 8

E4 = ml_dtypes.float8_e4m3

# power-of-2 quantization scales
SG = 256.0    # G
SQ = 8.0      # QT
ST_ = 8.0     # S^T (stage2 stationary)
ST__ = 16.0   # TMP
SE = 1.0      # expT (ln SE folds into the exp bias)
SSN = 8.0     # SN (stage3 stationary)
SA = 128.0    # AOSs
SH = 256.0    # HT

# compensation flags (both sides of every stage compensated by default)
COMP_QT = True
COMP_TMP = True

N_WARMUP = 50

_CACHE = {}


def _build_program():
    import concourse.mybir as mybir
    import concourse.tile as tile
    from concourse import bacc

    f32 = mybir.dt.float32
    f8 = mybir.dt.float8e4
    DR = mybir.MatmulPerfMode.DoubleRow
    Exp = mybir.ActivationFunctionType.Exp
    Identity = mybir.ActivationFunctionType.Identity
    MULT = mybir.AluOpType.mult
    SUB = mybir.AluOpType.subtract

    SCALE = 1.0 / float(np.sqrt(D))

    nc = bacc.Bacc(
        "TRN2", target_bir_lowering=False, debug=False, enable_asserts=False
    )

    def din(name, shape, dt=f8):
        return nc.dram_tensor(name, shape, dt, kind="ExternalInput").ap()

    # host-prepped fp8 pair tensors: [128 part, 4 pair, 2 plane, 1024 cols]
    QT_hi_d = din("QT_hi", [128, 4, 2, LQ])
    QT_lo_d = din("QT_lo", [128, 4, 2, LQ])
    G_hi_d = [din(f"G{p}_hi", [128, 4, 2, D]) for p in range(2)]
    G_lo_d = [din(f"G{p}_lo", [128, 4, 2, D]) for p in range(2)]
    ST_hi_d = [din(f"ST{p}_hi", [128, 4, 2, LK]) for p in range(2)]
    ST_lo_d = [din(f"ST{p}_lo", [128, 4, 2, LK]) for p in range(2)]
    SN_hi_d = [din(f"SN{p}_hi", [128, 4, 2, D]) for p in range(2)]
    SN_lo_d = [din(f"SN{p}_lo", [128, 4, 2, D]) for p in range(2)]
    HT_hi_d = [din(f"HT{p}_hi", [128, 4, 2, D]) for p in range(2)]
    HT_lo_d = [din(f"HT{p}_lo", [128, 4, 2, D]) for p in range(2)]
    vb_d = [din(f"vb{p}", [LK, 1], f32) for p in range(2)]
    boe_c = din("boe", [D, 1], f32)
    wgt = din("wgt", [1, 2], f32)  # w_p * SA / SSN
    ones_d = din("ones_pair", [128, 2, 16])
    outT = nc.dram_tensor("outT", [D, LQ], f32, kind="ExternalOutput").ap()

    with tile.TileContext(nc) as tc:
        import contextlib

        with contextlib.ExitStack() as ctx:
            const = ctx.enter_context(tc.tile_pool(name="const", bufs=1))
            warmp = ctx.enter_context(tc.tile_pool(name="warmp", bufs=1))
            qtp = ctx.enter_context(tc.tile_pool(name="qtp", bufs=1))
            gp = ctx.enter_context(tc.tile_pool(name="gp", bufs=1))
            stp = ctx.enter_context(tc.tile_pool(name="stp", bufs=1))
            snp = ctx.enter_context(tc.tile_pool(name="snp", bufs=1))
            htp = ctx.enter_context(tc.tile_pool(name="htp", bufs=1))
            tmpp = ctx.enter_context(tc.tile_pool(name="tmpp", bufs=1))
            expp = ctx.enter_context(tc.tile_pool(name="expp", bufs=1))
            aosp = ctx.enter_context(tc.tile_pool(name="aosp", bufs=1))
            scrp = ctx.enter_context(tc.tile_pool(name="scrp", bufs=3))
            vecp = ctx.enter_context(tc.tile_pool(name="vecp", bufs=2))
            sbcp = ctx.enter_context(tc.tile_pool(name="sbcp", bufs=2))
            osbp = ctx.enter_context(tc.tile_pool(name="osbp", bufs=4))
            psp = ctx.enter_context(tc.tile_pool(name="psp", bufs=8, space="PSUM"))
            dramp = ctx.enter_context(tc.tile_pool(name="dramp", bufs=2, space="DRAM"))

            # ---- warmup: keep PE busy through the p-state ramp while the
            # first input DMAs are in flight
            warm = warmp.tile([128, 2, 128], f8)
            nc.gpsimd.memset(warm[:], 0)
            wps_t = psp.tile([16, 128], f32, tag="acc", name="wps_t")
            for _ in range(N_WARMUP):
                nc.tensor.matmul(
                    wps_t[:], warm[:, :, 0:16], warm[:], start=True, stop=True,
                    perf_mode=DR,
                )

            # ---- input DMAs, ordered to match stage-1's j-outer consumption:
            # per pair j deliver (QT_hi half, G_hi, G_lo) so hi+lo terms of
            # pair j can run while pair j+1 streams
            qt_hi = qtp.tile([128, 4, 2, LQ], f8)
            qt_lo = qtp.tile([128, 4, 2, LQ], f8)
            g_hi = [None, None]
            g_lo = [None, None]
            g_hi[0] = gp.tile([128, 4, 2, D], f8, tag="ghi", name="g_hi0")
            g_lo[0] = gp.tile([128, 4, 2, D], f8, tag="glo", name="g_lo0")
            for j in range(4):
                nc.sync.dma_start(qt_hi[:, j, :, 0:512], QT_hi_d[:, j, :, 0:512])
                nc.sync.dma_start(g_hi[0][:, j], G_hi_d[0][:, j])
                nc.sync.dma_start(g_lo[0][:, j], G_lo_d[0][:, j])
            if COMP_QT:
                for j in range(4):
                    nc.sync.dma_start(qt_lo[:, j, :, 0:512], QT_lo_d[:, j, :, 0:512])
            for j in range(4):
                nc.sync.dma_start(
                    qt_hi[:, j, :, 512:1024], QT_hi_d[:, j, :, 512:1024]
                )
            if COMP_QT:
                for j in range(4):
                    nc.sync.dma_start(
                        qt_lo[:, j, :, 512:1024], QT_lo_d[:, j, :, 512:1024]
                    )

            # constants (small, needed by the first exp / rowsum)
            ones_t = const.tile([128, 2, 16], f8)
            nc.sync.dma_start(ones_t[:], ones_d[:])
            vb_t = [const.tile([128, 8], f32, name=f"vb_t{p}") for p in range(2)]
            for p in range(2):
                nc.sync.dma_start(
                    vb_t[p][:], vb_d[p].rearrange("(t p) o -> p (t o)", p=128)
                )
            boe_t = const.tile([128, 8], f32)
            nc.sync.dma_start(boe_t[:], boe_c.rearrange("(t p) o -> p (t o)", p=128))
            wgt_sb = const.tile([1, 2], f32)
            nc.sync.dma_start(wgt_sb[:], wgt[:])

            st_hi = [None, None]
            st_lo = [None, None]
            sn_hi = [None, None]
            sn_lo = [None, None]
            ht_hi = [None, None]
            ht_lo = [None, None]

            def load_pairs(pool, tag, name, dram):
                t = pool.tile([128, 4, 2, D], f8, tag=tag, name=name)
                nc.sync.dma_start(t[:], dram[:])
                return t

            # stage-2/3 operands for path 0 follow behind the stage-1 set
            st_hi[0] = load_pairs(stp, "sthi", "st_hi0", ST_hi_d[0])
            st_lo[0] = load_pairs(stp, "stlo", "st_lo0", ST_lo_d[0])
            sn_hi[0] = load_pairs(snp, "snhi", "sn_hi0", SN_hi_d[0])
            sn_lo[0] = load_pairs(snp, "snlo", "sn_lo0", SN_lo_d[0])

            aos_hi = [None, None]
            aos_lo = [None, None]
            sbc = [[None, None], [None, None]]

            for p in range(2):
                if p == 1:
                    # path-1 operands (slots freed by path 0 reuse via tags)
                    g_hi[1] = gp.tile([128, 4, 2, D], f8, tag="ghi", name="g_hi1")
                    nc.sync.dma_start(g_hi[1][:], G_hi_d[1][:])
                    g_lo[1] = gp.tile([128, 4, 2, D], f8, tag="glo", name="g_lo1")
                    nc.sync.dma_start(g_lo[1][:], G_lo_d[1][:])
                    st_hi[1] = load_pairs(stp, "sthi", "st_hi1", ST_hi_d[1])
                    st_lo[1] = load_pairs(stp, "stlo", "st_lo1", ST_lo_d[1])
                    sn_hi[1] = load_pairs(snp, "snhi", "sn_hi1", SN_hi_d[1])
                    sn_lo[1] = load_pairs(snp, "snlo", "sn_lo1", SN_lo_d[1])

                # =====================================================
                # S1: TMP[d',q] = sum_d G[d,d'] QT[d,q]   (DR pairs over d)
                # =====================================================
                tmp_hi = tmpp.tile([128, 4, 2, LQ], f8, tag="tmphi", name="tmp_hi")
                tmp_lo = tmpp.tile([128, 4, 2, LQ], f8, tag="tmplo", name="tmp_lo")
                # (j, term) consumption order matches the DMA delivery order:
                # pair j's hi+lo G terms run back-to-back, QT_lo term last
                s1_sched = [(j, t) for j in range(4) for t in range(2)]
                if COMP_QT:
                    s1_sched += [(j, 2) for j in range(4)]
                s1_ops = [(g_hi[p], qt_hi), (g_lo[p], qt_hi), (g_hi[p], qt_lo)]
                n_mm = len(s1_sched)
                def s1_cast(ps, dp, qs):
                    dst_hi = tmp_hi[:, dp // 2, dp % 2, qs]
                    nc.scalar.activation(
                        dst_hi, ps[:], Identity, scale=float(ST__ / (SG * SQ))
                    )
                    if COMP_TMP:
                        nc.vector.scalar_tensor_tensor(
                            tmp_lo[:, dp // 2, dp % 2, qs],
                            ps[:],
                            float(ST__ / (SG * SQ)),
                            dst_hi,
                            MULT,
                            SUB,
                        )

                if p == 0:
                    # q_b 0: (j, term)-outer — consumption tracks DMA delivery
                    qs = slice(0, 512)
                    ps_t = [
                        psp.tile([128, 512], f32, tag="acc", name="ps1")
                        for i in range(8)
                    ]
                    for k, (j, t) in enumerate(s1_sched):
                        gt, qt = s1_ops[t]
                        for dp in range(8):
                            nc.tensor.matmul(
                                ps_t[dp][:],
                                gt[:, j, :, dp * 128 : (dp + 1) * 128],
                                qt[:, j, :, qs],
                                start=(k == 0),
                                stop=(k == n_mm - 1),
                                perf_mode=DR,
                            )
                    for dp in range(8):
                        s1_cast(ps_t[dp], dp, qs)
                    qb1_list = [1]
                else:
                    qb1_list = [0, 1]

                # dp-outer — staggers PSUM bank release for pipelining
                for q_b1 in qb1_list:
                  qs = slice(q_b1 * 512, (q_b1 + 1) * 512)
                  for dp in range(8):
                    ps = psp.tile([128, 512], f32, tag="acc", name="ps1b")
                    for k, (j, t) in enumerate(s1_sched):
                        gt, qt = s1_ops[t]
                        nc.tensor.matmul(
                            ps[:],
                            gt[:, j, :, dp * 128 : (dp + 1) * 128],
                            qt[:, j, :, qs],
                            start=(k == 0),
                            stop=(k == n_mm - 1),
                            perf_mode=DR,
                        )
                    s1_cast(ps, dp, qs)
                del qb1_list

                # =====================================================
                # S2: logitsT[k,q] = sum_d' ST[d',k] TMP[d',q] ; exp + hi/lo
                # =====================================================
                exp_hi = expp.tile([128, 4, 2, LQ], f8, tag="exphi", name="exp_hi")
                exp_lo = expp.tile([128, 4, 2, LQ], f8, tag="explo", name="exp_lo")
                s2_terms = [(st_hi[p], tmp_hi), (st_lo[p], tmp_hi)]
                if COMP_TMP:
                    s2_terms.append((st_hi[p], tmp_lo))
                for q_b in range(2):
                    qs = slice(q_b * 512, (q_b + 1) * 512)
                    for k_t in range(8):
                        ps = psp.tile([128, 512], f32, tag="acc", name="ps2")
                        n_mm = len(s2_terms) * 4
                        k = 0
                        for (st_, tm) in s2_terms:
                            for j in range(4):
                                nc.tensor.matmul(
                                    ps[:],
                                    st_[:, j, :, k_t * 128 : (k_t + 1) * 128],
                                    tm[:, j, :, qs],
                                    start=(k == 0),
                                    stop=(k == n_mm - 1),
                                    perf_mode=DR,
                                )
                                k += 1
                        scr = scrp.tile([128, 512], f32, tag="scr", name="scr")
                        nc.scalar.activation(
                            scr[:], ps[:], Exp,
                            bias=vb_t[p][:, k_t : k_t + 1],
                            scale=float(SCALE / (ST_ * ST__)),
                        )
                        dst_hi = exp_hi[:, k_t // 2, k_t % 2, qs]
                        nc.scalar.activation(dst_hi, scr[:], Identity)
                        lo_eng = nc.gpsimd if k_t % 2 == 0 else nc.vector
                        lo_eng.tensor_tensor(
                            exp_lo[:, k_t // 2, k_t % 2, qs], scr[:], dst_hi, SUB
                        )

                # rowsum over quantized exp (hi+lo) via DR ones-matmul,
                # then sbc = w * SA/SSN / rowsum broadcast over partitions.
                # Emission is deferred behind covering PE work so the ones-
                # matmuls never stall on the Pool exp_lo chain.
                def emit_rowsum(q_b):
                    qs = slice(q_b * 512, (q_b + 1) * 512)
                    ps_s = psp.tile([1, 512], f32, tag="acc", name="ps_s")
                    k = 0
                    for et in (exp_hi, exp_lo):
                        for j in range(4):
                            nc.tensor.matmul(
                                ps_s[:],
                                ones_t[:, :, 0:1],
                                et[:, j, :, qs],
                                start=(k == 0),
                                stop=(k == 7),
                                perf_mode=DR,
                            )
                            k += 1
                    rs = vecp.tile([1, 512], f32, tag="rs", name="rs")
                    nc.vector.reciprocal(rs[:], ps_s[:])
                    s_row = vecp.tile([1, 512], f32, tag="srow", name="s_row")
                    nc.vector.tensor_scalar_mul(
                        s_row[:], rs[:], wgt_sb[0:1, p : p + 1]
                    )
                    srow_d = dramp.tile([1, 512], f32, tag="srd", name="srow_d")
                    nc.sync.dma_start(srow_d[:], s_row[:])
                    sb_t = sbcp.tile([128, 512], f32, tag=f"sbc{q_b}", name="sb_t")
                    nc.sync.dma_start(sb_t[:], srow_d[0:1, :].partition_broadcast(128))
                    sbc[p][q_b] = sb_t

                emit_rowsum(0)

                if p == 0:
                    # path-1 HT prefetch slot behind path-0 compute
                    ht_hi[0] = load_pairs(htp, "hthi0", "ht_hi0", HT_hi_d[0])
                    ht_lo[0] = load_pairs(htp, "htlo0", "ht_lo0", HT_lo_d[0])

                # =====================================================
                # S3: AOS[d',q] = sum_k SN[k,d'] expT[k,q] ; scale + hi/lo
                # =====================================================
                aos_hi[p] = aosp.tile(
                    [128, 4, 2, LQ], f8, tag=f"aoshi{p}", name=f"aos_hi{p}"
                )
                aos_lo[p] = aosp.tile(
                    [128, 4, 2, LQ], f8, tag=f"aoslo{p}", name=f"aos_lo{p}"
                )
                s3_terms = [(sn_hi[p], exp_hi), (sn_lo[p], exp_hi), (sn_hi[p], exp_lo)]

                def emit_s3(q_b, dps):
                    qs = slice(q_b * 512, (q_b + 1) * 512)
                    for dp in dps:
                        ps = psp.tile([128, 512], f32, tag="acc", name="ps3")
                        k = 0
                        for (sn, et) in s3_terms:
                            for j in range(4):
                                nc.tensor.matmul(
                                    ps[:],
                                    sn[:, j, :, dp * 128 : (dp + 1) * 128],
                                    et[:, j, :, qs],
                                    start=(k == 0),
                                    stop=(k == 11),
                                    perf_mode=DR,
                                )
                                k += 1
                        t32 = scrp.tile([128, 512], f32, tag="scr", name="t32")
                        nc.vector.tensor_tensor(t32[:], ps[:], sbc[p][q_b][:], MULT)
                        dst_hi = aos_hi[p][:, dp // 2, dp % 2, qs]
                        nc.vector.tensor_copy(dst_hi, t32[:])
                        lo_eng = nc.gpsimd if dp % 2 == 0 else nc.vector
                        lo_eng.tensor_tensor(
                            aos_lo[p][:, dp // 2, dp % 2, qs], t32[:], dst_hi, SUB
                        )

                emit_s3(0, range(0, 4))
                emit_rowsum(1)
                emit_s3(0, range(4, 8))
                emit_s3(1, range(0, 8))

                if p == 0:
                    ht_hi[1] = load_pairs(htp, "hthi1", "ht_hi1", HT_hi_d[1])
                    ht_lo[1] = load_pairs(htp, "htlo1", "ht_lo1", HT_lo_d[1])

            # =====================================================
            # S4: outT[o,q] = sum_p sum_d' HT_p[d',o] AOSs_p[d',q]
            # single accumulation over both paths (contraction 2048 x 3 terms)
            # =====================================================
            s4_terms = [
                (p, ht, at)
                for p in range(2)
                for (ht, at) in (
                    (ht_hi[p], aos_hi[p]),
                    (ht_lo[p], aos_hi[p]),
                    (ht_hi[p], aos_lo[p]),
                )
            ]

            def emit_s4_chain(o_t, q0, width):
                ps = psp.tile([128, width], f32, tag="acc", name="ps4")
                qsl = slice(q0, q0 + width)
                k = 0
                for (_, ht, at) in s4_terms:
                    for j in range(4):
                        nc.tensor.matmul(
                            ps[:],
                            ht[:, j, :, o_t * 128 : (o_t + 1) * 128],
                            at[:, j, :, qsl],
                            start=(k == 0),
                            stop=(k == 23),
                            perf_mode=DR,
                        )
                        k += 1
                osb = osbp.tile([128, width], f32, tag="osb", name="osb")
                nc.scalar.activation(
                    osb[:], ps[:], Identity,
                    bias=boe_t[:, o_t : o_t + 1],
                    scale=float(1.0 / (SH * SA)),
                )
                nc.sync.dma_start(
                    outT[o_t * 128 : (o_t + 1) * 128, qsl], osb[:]
                )

            for q_b in range(2):
                for o_t in range(8):
                    if q_b == 1 and o_t == 7:
                        # split the last tile so the final copy+DMA chain is
                        # short and pipelines behind the preceding matmuls
                        emit_s4_chain(o_t, 512, 256)
                        emit_s4_chain(o_t, 768, 128)
                        emit_s4_chain(o_t, 896, 128)
                    else:
                        emit_s4_chain(o_t, q_b * 512, 512)

    nc.compile()
    return nc


def _get_program():
    if "nc" not in _CACHE:
        _CACHE["nc"] = _build_program()
    return _CACHE["nc"]


def _host_gating(Q, Wq, bq, Wm1, bm1, Wm2, bm2):
    """Replicates the reference path-score MLP + top-k sparse weights."""
    Qm = Q.astype(np.float64).mean(axis=1)  # [B, D]
    pooled = Qm @ Wq.astype(np.float64).T + bq.astype(np.float64)
    h = np.maximum(pooled @ Wm1.astype(np.float64).T + bm1.astype(np.float64), 0.0)
    pl = h @ Wm2.astype(np.float64).T + bm2.astype(np.float64)  # [B, P]
    pl = pl - pl.max(axis=1, keepdims=True)
    e = np.exp(pl)
    scores = e / e.sum(axis=1, keepdims=True)
    idx = np.argsort(-scores, axis=1, kind="stable")[:, :TOP_K]  # [B, 2]
    w = np.take_along_axis(scores, idx, axis=1)
    wn = w / (w.sum(axis=1, keepdims=True) + 1e-8)
    return idx.astype(np.int64), wn.astype(np.float32)


def _q8_pair(x, scale):
    """x [1024, C] -> (hi, lo) fp8 pair tensors [128, 4, 2, C]."""
    xs = (np.asarray(x, np.float32) * np.float32(scale)).astype(np.float32)
    hi = xs.astype(E4)
    lo = (xs - hi.astype(np.float32)).astype(E4)
    C = x.shape[1]

    def lay(a):
        return np.ascontiguousarray(
            a.reshape(4, 2, 128, C).transpose(2, 0, 1, 3)
        )

    return lay(hi), lay(lo)


def kernel(**inputs):
    from concourse.bass_utils import run_bass_kernel_spmd

    Q = np.asarray(inputs["Q"], dtype=np.float32)
    src = np.asarray(inputs["src"], dtype=np.float32)
    Wq = np.asarray(inputs["Wq"], dtype=np.float32)
    bq = np.asarray(inputs["bq"], dtype=np.float32)
    Wk = np.asarray(inputs["Wk"], dtype=np.float32)
    Wv = np.asarray(inputs["Wv"], dtype=np.float32)
    bv = np.asarray(inputs["bv"], dtype=np.float32)
    Wm1 = np.asarray(inputs["Wm1"], dtype=np.float32)
    bm1 = np.asarray(inputs["bm1"], dtype=np.float32)
    Wm2 = np.asarray(inputs["Wm2"], dtype=np.float32)
    bm2 = np.asarray(inputs["bm2"], dtype=np.float32)
    Wo = np.asarray(inputs["Wo"], dtype=np.float32)
    bo = np.asarray(inputs["bo"], dtype=np.float32)

    idx, wn = _host_gating(Q, Wq, bq, Wm1, bm1, Wm2, bm2)
    SCALE = 1.0 / float(np.sqrt(D))

    nc = _get_program()

    # host-folded weights, shared across cores (<=4 selected paths)
    sel = sorted(set(idx.flatten().tolist()))
    WqT = Wq.T
    G8 = {p: _q8_pair(WqT @ Wk[p], SG) for p in sel}
    HT8 = {p: _q8_pair((Wo @ Wv[p]).T, SH) for p in sel}
    g2 = {p: Wk[p].T @ bq for p in sel}
    Wobv = {p: Wo @ bv[p] for p in sel}
    ones_pair = np.ones((128, 2, 16), dtype=E4)
    LN_SE = float(np.log(SE))

    in_maps = []
    for b in range(B):
        p0, p1 = int(idx[b, 0]), int(idx[b, 1])
        boe = bo + wn[b, 0] * Wobv[p0] + wn[b, 1] * Wobv[p1]
        qt_hi, qt_lo = _q8_pair(Q[b].T, SQ)
        m = {
            "QT_hi": qt_hi,
            "QT_lo": qt_lo,
            "boe": np.ascontiguousarray(boe.reshape(D, 1).astype(np.float32)),
            "wgt": np.ascontiguousarray(
                (wn[b] * (SA / SSN)).reshape(1, 2).astype(np.float32)
            ),
            "ones_pair": ones_pair,
        }
        for i, p in enumerate((p0, p1)):
            S = src[p, b]
            m[f"G{i}_hi"], m[f"G{i}_lo"] = G8[p]
            m[f"ST{i}_hi"], m[f"ST{i}_lo"] = _q8_pair(S.T, ST_)
            m[f"SN{i}_hi"], m[f"SN{i}_lo"] = _q8_pair(S, SSN)
            m[f"HT{i}_hi"], m[f"HT{i}_lo"] = HT8[p]
            m[f"vb{i}"] = np.ascontiguousarray(
                ((S @ g2[p]) * SCALE + LN_SE).reshape(LK, 1).astype(np.float32)
            )
        in_maps.append(m)

    res = run_bass_kernel_spmd(nc, in_maps, core_ids=list(range(N_CORES)))
    out = np.stack([res.results[b]["outT"].T for b in range(B)], axis=0)
    return np.ascontiguousarray(out).astype(np.float32)


# revision 19
# speedup vs baseline: 1.3587x; 1.0087x over previous
"""Trainium2 Bass kernel for DynamicPathCrossAttention.

Sharding: batch-parallel — core b computes batch element b end-to-end. The
path-gating MLP runs on the host; each core computes cross-attention for its
batch element's TOP_K=2 selected paths only.

Weight folding (host, shared across cores): the chain is linear around the
softmax, so adjacent projection pairs collapse:
  logits = Q Wq^T Wk S^T          -> G_p = Wq^T @ Wk_p     (logits = Q G S^T)
  out    = attn S Wv^T Wo^T (...) -> H_p = Wo @ Wv_p       (out = attn S H^T)
Bias algebra: per-q logit terms cancel in softmax; the per-k term ships as an
exp() bias column vb; bv folds into an effective output bias boe.

All matmuls run as fp8e4 (e4m3) DoubleRow with hi/lo error compensation:
every operand X is split on host (or on device for intermediates) into
  X_hi = fp8(X*s),  X_lo = fp8(X*s - X_hi)
and each contraction A@B is computed as A_hi@B_hi + A_lo@B_hi + A_hi@B_lo
(the lo*lo term ~eps^2 is dropped), keeping rel err ~2e-3 at 2x the f32r
matmul rate. DoubleRow packs two 128-row contraction planes per matmul:
operands are laid out [128 part, 2 planes, cols]; logical contraction index
d = 256*j + 128*i + p for pair-tile j, plane i, partition p.

Per-core pipeline (scales are powers of 2, folded into ACT scale factors):
  S1: TMP[d',q] = G^T QT      (G,QT host fp8 pairs; TMP hi/lo via ACT+DVE)
  S2: logitsT[k,q] = ST^T TMP ; expT = exp(.*SCALE + vb + ln sE) (ACT, f32)
      exp hi cast (ACT) + lo residual (Pool tensor_tensor sub)
  rowsum = DR ones-matmul over exp hi+lo pairs; sbc = w/rowsum broadcast
      (per-q) via DRAM-bounce partition_broadcast
  S3: AOS[d',q] = SN^T expT ; AOSs = AOS * sbc (DVE), hi cast (DVE) +
      lo residual (Pool)
  S4 (both paths in one accumulation, contraction 2048):
      outT[o,q] = sum_p HT_p^T AOSs_p ; final ACT copy applies 2^-19 + boe.

A memset-fed warmup matmul chain keeps the PE busy from t~1us so the
p-state ramp completes before the first real matmul (DMA-latency window).
"""

import numpy as np
import ml_dtypes

D = 1024
P = 4
TOP_K = 2
B = 8
LQ = 1024
LK = 1024
N_CORES = 8

E4 = ml_dtypes.float8_e4m3

# power-of-2 quantization scales
SG = 256.0    # G
SQ = 8.0      # QT
ST_ = 8.0     # S^T (stage2 stationary)
ST__ = 16.0   # TMP
SE = 1.0      # expT (ln SE folds into the exp bias)
SSN = 8.0     # SN (stage3 stationary)
SA = 128.0    # AOSs
SH = 256.0    # HT

# compensation flags (both sides of every stage compensated by default)
COMP_QT = True
COMP_TMP = True

N_WARMUP = 50

_CACHE = {}


def _build_program():
    import concourse.mybir as mybir
    import concourse.tile as tile
    from concourse import bacc

    f32 = mybir.dt.float32
    f8 = mybir.dt.float8e4
    DR = mybir.MatmulPerfMode.DoubleRow
    Exp = mybir.ActivationFunctionType.Exp
    Identity = mybir.ActivationFunctionType.Identity
    MULT = mybir.AluOpType.mult
    SUB = mybir.AluOpType.subtract

    SCALE = 1.0 / float(np.sqrt(D))

    nc = bacc.Bacc(
        "TRN2", target_bir_lowering=False, debug=False, enable_asserts=False
    )

    def din(name, shape, dt=f8):
        return nc.dram_tensor(name, shape, dt, kind="ExternalInput").ap()

    # host-prepped fp8 pair tensors: [128 part, 4 pair, 2 plane, 1024 cols]
    QT_hi_d = din("QT_hi", [128, 4, 2, LQ])
    QT_lo_d = din("QT_lo", [128, 4, 2, LQ])
    G_pair_d = [din(f"G{p}_pair", [128, 4, 2, 2 * D]) for p in range(2)]
    ST_hi_d = [din(f"ST{p}_hi", [128, 4, 2, LK]) for p in range(2)]
    ST_lo_d = [din(f"ST{p}_lo", [128, 4, 2, LK]) for p in range(2)]
    SN_hi_d = [din(f"SN{p}_hi", [128, 4, 2, D]) for p in range(2)]
    SN_lo_d = [din(f"SN{p}_lo", [128, 4, 2, D]) for p in range(2)]
    HT_hi_d = [din(f"HT{p}_hi", [128, 4, 2, D]) for p in range(2)]
    HT_lo_d = [din(f"HT{p}_lo", [128, 4, 2, D]) for p in range(2)]
    vb_d = [din(f"vb{p}", [LK, 1], f32) for p in range(2)]
    boe_c = din("boe", [D, 1], f32)
    wgt = din("wgt", [1, 2], f32)  # w_p * SA / SSN
    ones_d = din("ones_pair", [128, 2, 16])
    outT = nc.dram_tensor("outT", [D, LQ], f32, kind="ExternalOutput").ap()

    with tile.TileContext(nc) as tc:
        import contextlib

        with contextlib.ExitStack() as ctx:
            const = ctx.enter_context(tc.tile_pool(name="const", bufs=1))
            warmp = ctx.enter_context(tc.tile_pool(name="warmp", bufs=1))
            qtp = ctx.enter_context(tc.tile_pool(name="qtp", bufs=1))
            gp = ctx.enter_context(tc.tile_pool(name="gp", bufs=1))
            stp = ctx.enter_context(tc.tile_pool(name="stp", bufs=1))
            snp = ctx.enter_context(tc.tile_pool(name="snp", bufs=1))
            htp = ctx.enter_context(tc.tile_pool(name="htp", bufs=1))
            tmpp = ctx.enter_context(tc.tile_pool(name="tmpp", bufs=1))
            expp = ctx.enter_context(tc.tile_pool(name="expp", bufs=1))
            aosp = ctx.enter_context(tc.tile_pool(name="aosp", bufs=1))
            scrp = ctx.enter_context(tc.tile_pool(name="scrp", bufs=3))
            vecp = ctx.enter_context(tc.tile_pool(name="vecp", bufs=2))
            sbcp = ctx.enter_context(tc.tile_pool(name="sbcp", bufs=2))
            osbp = ctx.enter_context(tc.tile_pool(name="osbp", bufs=4))
            psp = ctx.enter_context(tc.tile_pool(name="psp", bufs=8, space="PSUM"))
            dramp = ctx.enter_context(tc.tile_pool(name="dramp", bufs=2, space="DRAM"))

            # ---- warmup: keep PE busy through the p-state ramp while the
            # first input DMAs are in flight
            warm = warmp.tile([128, 2, 128], f8)
            nc.vector.memset(warm[:], 0)
            wps_t = psp.tile([16, 128], f32, tag="acc", name="wps_t")
            for _ in range(N_WARMUP):
                nc.tensor.matmul(
                    wps_t[:], warm[:, :, 0:16], warm[:], start=True, stop=True,
                    perf_mode=DR,
                )

            # ---- input DMAs, ordered to match stage-1's j-outer consumption:
            # per pair j deliver (QT_hi half, G_hi, G_lo) so hi+lo terms of
            # pair j can run while pair j+1 streams
            qt_hi = qtp.tile([128, 4, 2, LQ], f8)
            qt_lo = qtp.tile([128, 4, 2, LQ], f8)
            g_pair = [None, None]
            g_pair[0] = gp.tile([128, 4, 2, 2 * D], f8, tag="gpair", name="g_pair0")
            for j in range(4):
                nc.sync.dma_start(qt_hi[:, j, :, 0:512], QT_hi_d[:, j, :, 0:512])
                nc.sync.dma_start(g_pair[0][:, j], G_pair_d[0][:, j])
            if COMP_QT:
                nc.sync.dma_start(qt_lo[:, :, :, 0:512], QT_lo_d[:, :, :, 0:512])
            nc.sync.dma_start(qt_hi[:, :, :, 512:1024], QT_hi_d[:, :, :, 512:1024])
            if COMP_QT:
                nc.sync.dma_start(
                    qt_lo[:, :, :, 512:1024], QT_lo_d[:, :, :, 512:1024]
                )

            # vb before the stage-2 operands (needed at the first exp)
            vb_t = [const.tile([128, 8], f32, name=f"vb_t{p}") for p in range(2)]
            for pp in range(2):
                nc.sync.dma_start(
                    vb_t[pp][:], vb_d[pp].rearrange("(t p) o -> p (t o)", p=128)
                )

            st_hi = [None, None]
            st_lo = [None, None]
            sn_hi = [None, None]
            sn_lo = [None, None]
            ht_hi = [None, None]
            ht_lo = [None, None]

            def load_pairs(pool, tag, name, dram):
                t = pool.tile([128, 4, 2, D], f8, tag=tag, name=name)
                nc.sync.dma_start(t[:], dram[:])
                return t

            # stage-2/3 operands for path 0 follow behind the stage-1 set
            st_hi[0] = load_pairs(stp, "sthi", "st_hi0", ST_hi_d[0])
            st_lo[0] = load_pairs(stp, "stlo", "st_lo0", ST_lo_d[0])

            # remaining small constants (rowsum / sbc / S4)
            ones_t = const.tile([128, 2, 16], f8)
            nc.sync.dma_start(ones_t[:], ones_d[:])
            wgt_sb = const.tile([1, 2], f32)
            nc.sync.dma_start(wgt_sb[:], wgt[:])
            boe_t = const.tile([128, 8], f32)
            nc.sync.dma_start(boe_t[:], boe_c.rearrange("(t p) o -> p (t o)", p=128))

            sn_hi[0] = load_pairs(snp, "snhi", "sn_hi0", SN_hi_d[0])
            sn_lo[0] = load_pairs(snp, "snlo", "sn_lo0", SN_lo_d[0])

            aos_hi = [None, None]
            aos_lo = [None, None]
            sbc = [[None, None], [None, None]]

            for p in range(2):
                if p == 1:
                    # path-1 operands (slots freed by path 0 reuse via tags)
                    g_pair[1] = gp.tile(
                        [128, 4, 2, 2 * D], f8, tag="gpair", name="g_pair1"
                    )
                    nc.sync.dma_start(g_pair[1][:], G_pair_d[1][:])
                    st_hi[1] = load_pairs(stp, "sthi", "st_hi1", ST_hi_d[1])
                    st_lo[1] = load_pairs(stp, "stlo", "st_lo1", ST_lo_d[1])
                    sn_hi[1] = load_pairs(snp, "snhi", "sn_hi1", SN_hi_d[1])
                    sn_lo[1] = load_pairs(snp, "snlo", "sn_lo1", SN_lo_d[1])

                # =====================================================
                # S1: TMP[d',q] = sum_d G[d,d'] QT[d,q]   (DR pairs over d)
                # =====================================================
                tmp_hi = [
                    tmpp.tile([128, 4, 2, 512], f8, tag=f"tmphi{qb}", name="tmp_hi")
                    for qb in range(2)
                ]
                tmp_lo = [
                    tmpp.tile([128, 4, 2, 512], f8, tag=f"tmplo{qb}", name="tmp_lo")
                    for qb in range(2)
                ]
                # (j, term) consumption order matches the DMA delivery order:
                # pair j's hi+lo G terms run back-to-back, QT_lo term last
                s1_sched = [(j, t) for j in range(4) for t in range(2)]
                if COMP_QT:
                    s1_sched += [(j, 2) for j in range(4)]
                s1_ops = [(0, qt_hi), (D, qt_hi), (0, qt_lo)]
                n_mm = len(s1_sched)
                def s1_cast(ps, dp, qb):
                    dst_hi = tmp_hi[qb][:, dp // 2, dp % 2, :]
                    nc.scalar.activation(
                        dst_hi, ps[:], Identity, scale=float(ST__ / (SG * SQ))
                    )
                    if COMP_TMP:
                        nc.vector.scalar_tensor_tensor(
                            tmp_lo[qb][:, dp // 2, dp % 2, :],
                            ps[:],
                            float(ST__ / (SG * SQ)),
                            dst_hi,
                            MULT,
                            SUB,
                        )

                if p == 0:
                    # (j, term)-outer in 4-dp halves — consumption tracks DMA
                    # delivery (lo terms last); each half's PSUM banks drain
                    # while the next half runs
                    for q_b0 in range(2):
                        qs = slice(q_b0 * 512, (q_b0 + 1) * 512)
                        for half in range(2):
                            dps = range(half * 4, half * 4 + 4)
                            ps_t = {
                                dp: psp.tile(
                                    [128, 512], f32, tag="acc", name="ps1"
                                )
                                for dp in dps
                            }
                            for k, (j, t) in enumerate(s1_sched):
                                off, qt = s1_ops[t]
                                for dp in dps:
                                    nc.tensor.matmul(
                                        ps_t[dp][:],
                                        g_pair[p][
                                            :, j, :,
                                            off + dp * 128 : off + (dp + 1) * 128,
                                        ],
                                        qt[:, j, :, qs],
                                        start=(k == 0),
                                        stop=(k == n_mm - 1),
                                        perf_mode=DR,
                                    )
                            for dp in dps:
                                s1_cast(ps_t[dp], dp, q_b0)
                    qb1_list = []
                else:
                    qb1_list = [0, 1]

                # dp-outer — staggers PSUM bank release for pipelining
                for q_b1 in qb1_list:
                  qs = slice(q_b1 * 512, (q_b1 + 1) * 512)
                  for dp in range(8):
                    ps = psp.tile([128, 512], f32, tag="acc", name="ps1b")
                    for k, (j, t) in enumerate(s1_sched):
                        off, qt = s1_ops[t]
                        nc.tensor.matmul(
                            ps[:],
                            g_pair[p][
                                :, j, :, off + dp * 128 : off + (dp + 1) * 128
                            ],
                            qt[:, j, :, qs],
                            start=(k == 0),
                            stop=(k == n_mm - 1),
                            perf_mode=DR,
                        )
                    s1_cast(ps, dp, q_b1)
                del qb1_list

                # =====================================================
                # S2: logitsT[k,q] = sum_d' ST[d',k] TMP[d',q] ; exp + hi/lo
                # =====================================================
                exp_hi = [
                    expp.tile([128, 4, 2, 512], f8, tag=f"exphi{qb}", name="exp_hi")
                    for qb in range(2)
                ]
                exp_lo = [
                    expp.tile([128, 4, 2, 512], f8, tag=f"explo{qb}", name="exp_lo")
                    for qb in range(2)
                ]
                s2_terms = [(st_hi[p], tmp_hi), (st_lo[p], tmp_hi)]
                if COMP_TMP:
                    s2_terms.append((st_hi[p], tmp_lo))
                for q_b in range(2):
                    for k_t in range(8):
                        ps = psp.tile([128, 512], f32, tag="acc", name="ps2")
                        n_mm = len(s2_terms) * 4
                        k = 0
                        for (st_, tm) in s2_terms:
                            for j in range(4):
                                nc.tensor.matmul(
                                    ps[:],
                                    st_[:, j, :, k_t * 128 : (k_t + 1) * 128],
                                    tm[q_b][:, j, :, :],
                                    start=(k == 0),
                                    stop=(k == n_mm - 1),
                                    perf_mode=DR,
                                )
                                k += 1
                        scr = scrp.tile([128, 512], f32, tag="scr", name="scr")
                        nc.scalar.activation(
                            scr[:], ps[:], Exp,
                            bias=vb_t[p][:, k_t : k_t + 1],
                            scale=float(SCALE / (ST_ * ST__)),
                        )
                        dst_hi = exp_hi[q_b][:, k_t // 2, k_t % 2, :]
                        nc.scalar.activation(dst_hi, scr[:], Identity)
                        lo_eng = nc.gpsimd if k_t % 2 == 0 else nc.vector
                        lo_eng.tensor_tensor(
                            exp_lo[q_b][:, k_t // 2, k_t % 2, :], scr[:], dst_hi, SUB
                        )

                # rowsum over quantized exp (hi+lo) via DR ones-matmul,
                # then sbc = w * SA/SSN / rowsum broadcast over partitions.
                # Emission is deferred behind covering PE work so the ones-
                # matmuls never stall on the Pool exp_lo chain.
                def emit_rowsum(q_b):
                    # sum of exp_hi only: the exp_lo column-sum is a zero-mean
                    # ~0.06% correction, far below the error budget
                    ps_s = psp.tile([1, 512], f32, tag="acc", name="ps_s")
                    for j in range(4):
                        nc.tensor.matmul(
                            ps_s[:],
                            ones_t[:, :, 0:1],
                            exp_hi[q_b][:, j, :, :],
                            start=(j == 0),
                            stop=(j == 3),
                            perf_mode=DR,
                        )
                    rs = vecp.tile([1, 512], f32, tag="rs", name="rs")
                    nc.vector.reciprocal(rs[:], ps_s[:])
                    s_row = vecp.tile([1, 512], f32, tag="srow", name="s_row")
                    nc.vector.tensor_scalar_mul(
                        s_row[:], rs[:], wgt_sb[0:1, p : p + 1]
                    )
                    srow_d = dramp.tile([1, 512], f32, tag="srd", name="srow_d")
                    nc.sync.dma_start(srow_d[:], s_row[:])
                    sb_t = sbcp.tile([128, 512], f32, tag=f"sbc{q_b}", name="sb_t")
                    nc.sync.dma_start(sb_t[:], srow_d[0:1, :].partition_broadcast(128))
                    sbc[p][q_b] = sb_t

                emit_rowsum(0)

                if p == 0:
                    # path-1 HT prefetch slot behind path-0 compute
                    ht_hi[0] = load_pairs(htp, "hthi0", "ht_hi0", HT_hi_d[0])
                    ht_lo[0] = load_pairs(htp, "htlo0", "ht_lo0", HT_lo_d[0])

                # =====================================================
                # S3: AOS[d',q] = sum_k SN[k,d'] expT[k,q] ; scale + hi/lo
                # =====================================================
                aos_hi[p] = [
                    aosp.tile([128, 4, 2, 512], f8, tag=f"aoshi{p}{qb}",
                              name=f"aos_hi{p}")
                    for qb in range(2)
                ]
                aos_lo[p] = [
                    aosp.tile([128, 4, 2, 512], f8, tag=f"aoslo{p}{qb}",
                              name=f"aos_lo{p}")
                    for qb in range(2)
                ]
                s3_terms = [(sn_hi[p], exp_hi), (sn_lo[p], exp_hi), (sn_hi[p], exp_lo)]

                def emit_s3(q_b, dps):
                    for dp in dps:
                        ps = psp.tile([128, 512], f32, tag="acc", name="ps3")
                        k = 0
                        for (sn, et) in s3_terms:
                            for j in range(4):
                                nc.tensor.matmul(
                                    ps[:],
                                    sn[:, j, :, dp * 128 : (dp + 1) * 128],
                                    et[q_b][:, j, :, :],
                                    start=(k == 0),
                                    stop=(k == 11),
                                    perf_mode=DR,
                                )
                                k += 1
                        t32 = scrp.tile([128, 512], f32, tag="scr", name="t32")
                        nc.vector.tensor_tensor(t32[:], ps[:], sbc[p][q_b][:], MULT)
                        dst_hi = aos_hi[p][q_b][:, dp // 2, dp % 2, :]
                        nc.vector.tensor_copy(dst_hi, t32[:])
                        lo_eng = nc.gpsimd if dp % 2 == 0 else nc.vector
                        lo_eng.tensor_tensor(
                            aos_lo[p][q_b][:, dp // 2, dp % 2, :], t32[:], dst_hi, SUB
                        )

                emit_s3(0, range(0, 4))
                emit_rowsum(1)
                emit_s3(0, range(4, 8))
                emit_s3(1, range(0, 8))

                if p == 0:
                    ht_hi[1] = load_pairs(htp, "hthi1", "ht_hi1", HT_hi_d[1])
                    ht_lo[1] = load_pairs(htp, "htlo1", "ht_lo1", HT_lo_d[1])

            # =====================================================
            # S4: outT[o,q] = sum_p sum_d' HT_p[d',o] AOSs_p[d',q]
            # single accumulation over both paths (contraction 2048 x 3 terms)
            # =====================================================
            s4_terms = [
                (p, ht, at)
                for p in range(2)
                for (ht, at) in (
                    (ht_hi[p], aos_hi[p]),
                    (ht_lo[p], aos_hi[p]),
                    (ht_hi[p], aos_lo[p]),
                )
            ]

            def emit_s4_chain(o_t, q0, width):
                ps = psp.tile([128, width], f32, tag="acc", name="ps4")
                q_b = q0 // 512
                qsl = slice(q0 - q_b * 512, q0 - q_b * 512 + width)
                k = 0
                for (_, ht, at) in s4_terms:
                    for j in range(4):
                        nc.tensor.matmul(
                            ps[:],
                            ht[:, j, :, o_t * 128 : (o_t + 1) * 128],
                            at[q_b][:, j, :, qsl],
                            start=(k == 0),
                            stop=(k == 23),
                            perf_mode=DR,
                        )
                        k += 1
                osb = osbp.tile([128, width], f32, tag="osb", name="osb")
                nc.scalar.activation(
                    osb[:], ps[:], Identity,
                    bias=boe_t[:, o_t : o_t + 1],
                    scale=float(1.0 / (SH * SA)),
                )
                nc.sync.dma_start(
                    outT[o_t * 128 : (o_t + 1) * 128, q0 : q0 + width], osb[:]
                )

            for q_b in range(2):
                for o_t in range(8):
                    if q_b == 1 and o_t == 7:
                        # split the last tile so the final copy+DMA chain is
                        # short and pipelines behind the preceding matmuls
                        emit_s4_chain(o_t, 512, 256)
                        emit_s4_chain(o_t, 768, 128)
                        emit_s4_chain(o_t, 896, 128)
                    else:
                        emit_s4_chain(o_t, q_b * 512, 512)

    nc.compile()
    return nc


def _get_program():
    if "nc" not in _CACHE:
        _CACHE["nc"] = _build_program()
    return _CACHE["nc"]


def _host_gating(Q, Wq, bq, Wm1, bm1, Wm2, bm2):
    """Replicates the reference path-score MLP + top-k sparse weights."""
    Qm = Q.astype(np.float64).mean(axis=1)  # [B, D]
    pooled = Qm @ Wq.astype(np.float64).T + bq.astype(np.float64)
    h = np.maximum(pooled @ Wm1.astype(np.float64).T + bm1.astype(np.float64), 0.0)
    pl = h @ Wm2.astype(np.float64).T + bm2.astype(np.float64)  # [B, P]
    pl = pl - pl.max(axis=1, keepdims=True)
    e = np.exp(pl)
    scores = e / e.sum(axis=1, keepdims=True)
    idx = np.argsort(-scores, axis=1, kind="stable")[:, :TOP_K]  # [B, 2]
    w = np.take_along_axis(scores, idx, axis=1)
    wn = w / (w.sum(axis=1, keepdims=True) + 1e-8)
    return idx.astype(np.int64), wn.astype(np.float32)


def _q8_pair(x, scale):
    """x [1024, C] -> (hi, lo) fp8 pair tensors [128, 4, 2, C]."""
    xs = (np.asarray(x, np.float32) * np.float32(scale)).astype(np.float32)
    hi = xs.astype(E4)
    lo = (xs - hi.astype(np.float32)).astype(E4)
    C = x.shape[1]

    def lay(a):
        return np.ascontiguousarray(
            a.reshape(4, 2, 128, C).transpose(2, 0, 1, 3)
        )

    return lay(hi), lay(lo)


def kernel(**inputs):
    from concourse.bass_utils import run_bass_kernel_spmd

    Q = np.asarray(inputs["Q"], dtype=np.float32)
    src = np.asarray(inputs["src"], dtype=np.float32)
    Wq = np.asarray(inputs["Wq"], dtype=np.float32)
    bq = np.asarray(inputs["bq"], dtype=np.float32)
    Wk = np.asarray(inputs["Wk"], dtype=np.float32)
    Wv = np.asarray(inputs["Wv"], dtype=np.float32)
    bv = np.asarray(inputs["bv"], dtype=np.float32)
    Wm1 = np.asarray(inputs["Wm1"], dtype=np.float32)
    bm1 = np.asarray(inputs["bm1"], dtype=np.float32)
    Wm2 = np.asarray(inputs["Wm2"], dtype=np.float32)
    bm2 = np.asarray(inputs["bm2"], dtype=np.float32)
    Wo = np.asarray(inputs["Wo"], dtype=np.float32)
    bo = np.asarray(inputs["bo"], dtype=np.float32)

    idx, wn = _host_gating(Q, Wq, bq, Wm1, bm1, Wm2, bm2)
    SCALE = 1.0 / float(np.sqrt(D))

    nc = _get_program()

    # host-folded weights, shared across cores (<=4 selected paths)
    sel = sorted(set(idx.flatten().tolist()))
    WqT = Wq.T
    G8 = {
        p: np.ascontiguousarray(np.concatenate(_q8_pair(WqT @ Wk[p], SG), axis=3))
        for p in sel
    }
    HT8 = {p: _q8_pair((Wo @ Wv[p]).T, SH) for p in sel}
    g2 = {p: Wk[p].T @ bq for p in sel}
    Wobv = {p: Wo @ bv[p] for p in sel}
    ones_pair = np.ones((128, 2, 16), dtype=E4)
    LN_SE = float(np.log(SE))

    in_maps = []
    for b in range(B):
        p0, p1 = int(idx[b, 0]), int(idx[b, 1])
        boe = bo + wn[b, 0] * Wobv[p0] + wn[b, 1] * Wobv[p1]
        qt_hi, qt_lo = _q8_pair(Q[b].T, SQ)
        m = {
            "QT_hi": qt_hi,
            "QT_lo": qt_lo,
            "boe": np.ascontiguousarray(boe.reshape(D, 1).astype(np.float32)),
            "wgt": np.ascontiguousarray(
                (wn[b] * (SA / SSN)).reshape(1, 2).astype(np.float32)
            ),
            "ones_pair": ones_pair,
        }
        for i, p in enumerate((p0, p1)):
            S = src[p, b]
            m[f"G{i}_pair"] = G8[p]
            m[f"ST{i}_hi"], m[f"ST{i}_lo"] = _q8_pair(S.T, ST_)
            m[f"SN{i}_hi"], m[f"SN{i}_lo"] = _q8_pair(S, SSN)
            m[f"HT{i}_hi"], m[f"HT{i}_lo"] = HT8[p]
            m[f"vb{i}"] = np.ascontiguousarray(
                ((S @ g2[p]) * SCALE + LN_SE).reshape(LK, 1).astype(np.float32)
            )
        in_maps.append(m)

    res = run_bass_kernel_spmd(nc, in_maps, core_ids=list(range(N_CORES)))
    out = np.stack([res.results[b]["outT"].T for b in range(B)], axis=0)
    return np.ascontiguousarray(out).astype(np.float32)


# revision 20
# speedup vs baseline: 1.3720x; 1.0098x over previous
"""Trainium2 Bass kernel for DynamicPathCrossAttention.

Sharding: batch-parallel — core b computes batch element b end-to-end. The
path-gating MLP runs on the host; each core computes cross-attention for its
batch element's TOP_K=2 selected paths only.

Weight folding (host, shared across cores): the chain is linear around the
softmax, so adjacent projection pairs collapse:
  logits = Q Wq^T Wk S^T          -> G_p = Wq^T @ Wk_p     (logits = Q G S^T)
  out    = attn S Wv^T Wo^T (...) -> H_p = Wo @ Wv_p       (out = attn S H^T)
Bias algebra: per-q logit terms cancel in softmax; the per-k term ships as an
exp() bias column vb; bv folds into an effective output bias boe.

All matmuls run as fp8e4 (e4m3) DoubleRow with hi/lo error compensation:
every operand X is split on host (or on device for intermediates) into
  X_hi = fp8(X*s),  X_lo = fp8(X*s - X_hi)
and each contraction A@B is computed as A_hi@B_hi + A_lo@B_hi + A_hi@B_lo
(the lo*lo term ~eps^2 is dropped), keeping rel err ~2e-3 at 2x the f32r
matmul rate. DoubleRow packs two 128-row contraction planes per matmul:
operands are laid out [128 part, 2 planes, cols]; logical contraction index
d = 256*j + 128*i + p for pair-tile j, plane i, partition p.

Per-core pipeline (scales are powers of 2, folded into ACT scale factors):
  S1: TMP[d',q] = G^T QT      (G,QT host fp8 pairs; TMP hi/lo via ACT+DVE)
  S2: logitsT[k,q] = ST^T TMP ; expT = exp(.*SCALE + vb + ln sE) (ACT, f32)
      exp hi cast (ACT) + lo residual (Pool tensor_tensor sub)
  rowsum = DR ones-matmul over exp_hi pairs (the exp_lo column-sum is a
      zero-mean ~0.1% correction, below the error budget); sbc = w/rowsum
      broadcast (per-q) via DRAM-bounce partition_broadcast
  S3: AOS[d',q] = SN^T expT ; AOSs = AOS * sbc (DVE), hi cast (DVE) +
      lo residual (Pool)
  S4 (both paths in one accumulation, contraction 2048):
      outT[o,q] = sum_p HT_p^T AOSs_p ; final ACT copy applies 2^-19 + boe.

A memset-fed warmup matmul chain keeps the PE busy from t~1us so the
p-state ramp completes before the first real matmul (DMA-latency window).
"""

import numpy as np
import ml_dtypes

D = 1024
P = 4
TOP_K = 2
B = 8
LQ = 1024
LK = 1024
N_CORES = 8

E4 = ml_dtypes.float8_e4m3

# power-of-2 quantization scales
SG = 256.0    # G
SQ = 8.0      # QT
ST_ = 8.0     # S^T (stage2 stationary)
ST__ = 16.0   # TMP
SE = 1.0      # expT (ln SE folds into the exp bias)
SSN = 8.0     # SN (stage3 stationary)
SA = 128.0    # AOSs
SH = 256.0    # HT

# compensation flags (both sides of every stage compensated by default)
COMP_QT = True
COMP_TMP = True

N_WARMUP = 50

_CACHE = {}


def _build_program():
    import concourse.mybir as mybir
    import concourse.tile as tile
    from concourse import bacc

    f32 = mybir.dt.float32
    f8 = mybir.dt.float8e4
    DR = mybir.MatmulPerfMode.DoubleRow
    Exp = mybir.ActivationFunctionType.Exp
    Identity = mybir.ActivationFunctionType.Identity
    MULT = mybir.AluOpType.mult
    SUB = mybir.AluOpType.subtract

    SCALE = 1.0 / float(np.sqrt(D))

    nc = bacc.Bacc(
        "TRN2", target_bir_lowering=False, debug=False, enable_asserts=False
    )

    def din(name, shape, dt=f8):
        return nc.dram_tensor(name, shape, dt, kind="ExternalInput").ap()

    # host-prepped fp8 pair tensors: [128 part, 4 pair, 2 plane, 1024 cols]
    QT_hi_d = din("QT_hi", [128, 4, 2, LQ])
    QT_lo_d = din("QT_lo", [128, 4, 2, LQ])
    G_pair_d = [din(f"G{p}_pair", [128, 4, 2, 2 * D]) for p in range(2)]
    ST_hi_d = [din(f"ST{p}_hi", [128, 4, 2, LK]) for p in range(2)]
    ST_lo_d = [din(f"ST{p}_lo", [128, 4, 2, LK]) for p in range(2)]
    SN_hi_d = [din(f"SN{p}_hi", [128, 4, 2, D]) for p in range(2)]
    SN_lo_d = [din(f"SN{p}_lo", [128, 4, 2, D]) for p in range(2)]
    HT_hi_d = [din(f"HT{p}_hi", [128, 4, 2, D]) for p in range(2)]
    HT_lo_d = [din(f"HT{p}_lo", [128, 4, 2, D]) for p in range(2)]
    vb_d = [din(f"vb{p}", [LK, 1], f32) for p in range(2)]
    boe_c = din("boe", [D, 1], f32)
    wgt = din("wgt", [1, 2], f32)  # w_p * SA / SSN
    ones_d = din("ones_pair", [128, 2, 16])
    outT = nc.dram_tensor("outT", [D, LQ], f32, kind="ExternalOutput").ap()

    with tile.TileContext(nc) as tc:
        import contextlib

        with contextlib.ExitStack() as ctx:
            const = ctx.enter_context(tc.tile_pool(name="const", bufs=1))
            warmp = ctx.enter_context(tc.tile_pool(name="warmp", bufs=1))
            qtp = ctx.enter_context(tc.tile_pool(name="qtp", bufs=1))
            gp = ctx.enter_context(tc.tile_pool(name="gp", bufs=1))
            stp = ctx.enter_context(tc.tile_pool(name="stp", bufs=1))
            snp = ctx.enter_context(tc.tile_pool(name="snp", bufs=1))
            htp = ctx.enter_context(tc.tile_pool(name="htp", bufs=1))
            tmpp = ctx.enter_context(tc.tile_pool(name="tmpp", bufs=1))
            expp = ctx.enter_context(tc.tile_pool(name="expp", bufs=1))
            aosp = ctx.enter_context(tc.tile_pool(name="aosp", bufs=1))
            scrp = ctx.enter_context(tc.tile_pool(name="scrp", bufs=3))
            vecp = ctx.enter_context(tc.tile_pool(name="vecp", bufs=2))
            sbcp = ctx.enter_context(tc.tile_pool(name="sbcp", bufs=2))
            osbp = ctx.enter_context(tc.tile_pool(name="osbp", bufs=4))
            psp = ctx.enter_context(tc.tile_pool(name="psp", bufs=8, space="PSUM"))
            dramp = ctx.enter_context(tc.tile_pool(name="dramp", bufs=2, space="DRAM"))

            # ---- warmup: keep PE busy through the p-state ramp while the
            # first input DMAs are in flight
            warm = warmp.tile([128, 2, 128], f8)
            nc.vector.memset(warm[:], 0)
            wps_t = psp.tile([16, 128], f32, tag="acc", name="wps_t")
            for _ in range(N_WARMUP):
                nc.tensor.matmul(
                    wps_t[:], warm[:, :, 0:16], warm[:], start=True, stop=True,
                    perf_mode=DR,
                )

            # ---- input DMAs, ordered to match stage-1's j-outer consumption:
            # per pair j deliver (QT_hi half, G_hi, G_lo) so hi+lo terms of
            # pair j can run while pair j+1 streams
            qt_hi = qtp.tile([128, 4, 2, LQ], f8)
            qt_lo = qtp.tile([128, 4, 2, LQ], f8)
            g_pair = [None, None]
            g_pair[0] = gp.tile([128, 4, 2, 2 * D], f8, tag="gpair", name="g_pair0")
            for j in range(4):
                nc.sync.dma_start(qt_hi[:, j, :, 0:512], QT_hi_d[:, j, :, 0:512])
                nc.sync.dma_start(g_pair[0][:, j], G_pair_d[0][:, j])
            if COMP_QT:
                nc.sync.dma_start(qt_lo[:, :, :, 0:512], QT_lo_d[:, :, :, 0:512])
            nc.sync.dma_start(qt_hi[:, :, :, 512:1024], QT_hi_d[:, :, :, 512:1024])
            if COMP_QT:
                nc.sync.dma_start(
                    qt_lo[:, :, :, 512:1024], QT_lo_d[:, :, :, 512:1024]
                )

            # vb before the stage-2 operands (needed at the first exp)
            vb_t = [const.tile([128, 8], f32, name=f"vb_t{p}") for p in range(2)]
            for pp in range(2):
                nc.sync.dma_start(
                    vb_t[pp][:], vb_d[pp].rearrange("(t p) o -> p (t o)", p=128)
                )

            st_hi = [None, None]
            st_lo = [None, None]
            sn_hi = [None, None]
            sn_lo = [None, None]
            ht_hi = [None, None]
            ht_lo = [None, None]

            def load_pairs(pool, tag, name, dram):
                t = pool.tile([128, 4, 2, D], f8, tag=tag, name=name)
                nc.sync.dma_start(t[:], dram[:])
                return t

            # stage-2/3 operands for path 0 follow behind the stage-1 set
            st_hi[0] = load_pairs(stp, "sthi", "st_hi0", ST_hi_d[0])
            st_lo[0] = load_pairs(stp, "stlo", "st_lo0", ST_lo_d[0])

            # remaining small constants (rowsum / sbc / S4)
            ones_t = const.tile([128, 2, 16], f8)
            nc.sync.dma_start(ones_t[:], ones_d[:])
            wgt_sb = const.tile([1, 2], f32)
            nc.sync.dma_start(wgt_sb[:], wgt[:])
            boe_t = const.tile([128, 8], f32)
            nc.sync.dma_start(boe_t[:], boe_c.rearrange("(t p) o -> p (t o)", p=128))

            sn_hi[0] = load_pairs(snp, "snhi", "sn_hi0", SN_hi_d[0])
            sn_lo[0] = load_pairs(snp, "snlo", "sn_lo0", SN_lo_d[0])

            aos_hi = [None, None]
            aos_lo = [None, None]
            sbc = [[None, None], [None, None]]

            for p in range(2):
                if p == 1:
                    # path-1 operands (slots freed by path 0 reuse via tags)
                    g_pair[1] = gp.tile(
                        [128, 4, 2, 2 * D], f8, tag="gpair", name="g_pair1"
                    )
                    nc.sync.dma_start(g_pair[1][:], G_pair_d[1][:])
                    st_hi[1] = load_pairs(stp, "sthi", "st_hi1", ST_hi_d[1])
                    st_lo[1] = load_pairs(stp, "stlo", "st_lo1", ST_lo_d[1])
                    sn_hi[1] = load_pairs(snp, "snhi", "sn_hi1", SN_hi_d[1])
                    sn_lo[1] = load_pairs(snp, "snlo", "sn_lo1", SN_lo_d[1])

                # =====================================================
                # S1: TMP[d',q] = sum_d G[d,d'] QT[d,q]   (DR pairs over d)
                # =====================================================
                tmp_hi = [
                    tmpp.tile([128, 4, 2, 512], f8, tag=f"tmphi{qb}", name="tmp_hi")
                    for qb in range(2)
                ]
                tmp_lo = [
                    tmpp.tile([128, 4, 2, 512], f8, tag=f"tmplo{qb}", name="tmp_lo")
                    for qb in range(2)
                ]
                # (j, term) consumption order matches the DMA delivery order:
                # pair j's hi+lo G terms run back-to-back, QT_lo term last
                s1_sched = [(j, t) for j in range(4) for t in range(2)]
                if COMP_QT:
                    s1_sched += [(j, 2) for j in range(4)]
                s1_ops = [(0, qt_hi), (D, qt_hi), (0, qt_lo)]
                n_mm = len(s1_sched)
                def s1_cast(ps, dp, qb):
                    dst_hi = tmp_hi[qb][:, dp // 2, dp % 2, :]
                    nc.scalar.activation(
                        dst_hi, ps[:], Identity, scale=float(ST__ / (SG * SQ))
                    )
                    if COMP_TMP:
                        nc.vector.scalar_tensor_tensor(
                            tmp_lo[qb][:, dp // 2, dp % 2, :],
                            ps[:],
                            float(ST__ / (SG * SQ)),
                            dst_hi,
                            MULT,
                            SUB,
                        )

                if p == 0:
                    # (j, term)-outer in 4-dp halves — consumption tracks DMA
                    # delivery (lo terms last); each half's PSUM banks drain
                    # while the next half runs
                    for q_b0 in range(2):
                        qs = slice(q_b0 * 512, (q_b0 + 1) * 512)
                        for half in range(2):
                            dps = range(half * 4, half * 4 + 4)
                            ps_t = {
                                dp: psp.tile(
                                    [128, 512], f32, tag="acc", name="ps1"
                                )
                                for dp in dps
                            }
                            for k, (j, t) in enumerate(s1_sched):
                                off, qt = s1_ops[t]
                                for dp in dps:
                                    nc.tensor.matmul(
                                        ps_t[dp][:],
                                        g_pair[p][
                                            :, j, :,
                                            off + dp * 128 : off + (dp + 1) * 128,
                                        ],
                                        qt[:, j, :, qs],
                                        start=(k == 0),
                                        stop=(k == n_mm - 1),
                                        perf_mode=DR,
                                    )
                            for dp in dps:
                                s1_cast(ps_t[dp], dp, q_b0)
                    qb1_list = []
                else:
                    qb1_list = [0, 1]

                # dp-outer — staggers PSUM bank release for pipelining
                for q_b1 in qb1_list:
                  qs = slice(q_b1 * 512, (q_b1 + 1) * 512)
                  for dp in range(8):
                    ps = psp.tile([128, 512], f32, tag="acc", name="ps1b")
                    for k, (j, t) in enumerate(s1_sched):
                        off, qt = s1_ops[t]
                        nc.tensor.matmul(
                            ps[:],
                            g_pair[p][
                                :, j, :, off + dp * 128 : off + (dp + 1) * 128
                            ],
                            qt[:, j, :, qs],
                            start=(k == 0),
                            stop=(k == n_mm - 1),
                            perf_mode=DR,
                        )
                    s1_cast(ps, dp, q_b1)
                del qb1_list

                # =====================================================
                # S2: logitsT[k,q] = sum_d' ST[d',k] TMP[d',q] ; exp + hi/lo
                # =====================================================
                exp_hi = [
                    expp.tile([128, 4, 2, 512], f8, tag=f"exphi{qb}", name="exp_hi")
                    for qb in range(2)
                ]
                exp_lo = [
                    expp.tile([128, 4, 2, 512], f8, tag=f"explo{qb}", name="exp_lo")
                    for qb in range(2)
                ]
                s2_terms = [(st_hi[p], tmp_hi), (st_lo[p], tmp_hi)]
                if COMP_TMP:
                    s2_terms.append((st_hi[p], tmp_lo))
                for q_b in range(2):
                    for k_t in range(8):
                        ps = psp.tile([128, 512], f32, tag="acc", name="ps2")
                        n_mm = len(s2_terms) * 4
                        k = 0
                        for (st_, tm) in s2_terms:
                            for j in range(4):
                                nc.tensor.matmul(
                                    ps[:],
                                    st_[:, j, :, k_t * 128 : (k_t + 1) * 128],
                                    tm[q_b][:, j, :, :],
                                    start=(k == 0),
                                    stop=(k == n_mm - 1),
                                    perf_mode=DR,
                                )
                                k += 1
                        scr = scrp.tile([128, 512], f32, tag="scr", name="scr")
                        nc.scalar.activation(
                            scr[:], ps[:], Exp,
                            bias=vb_t[p][:, k_t : k_t + 1],
                            scale=float(SCALE / (ST_ * ST__)),
                        )
                        dst_hi = exp_hi[q_b][:, k_t // 2, k_t % 2, :]
                        nc.scalar.activation(dst_hi, scr[:], Identity)
                        lo_eng = nc.gpsimd if k_t % 2 == 0 else nc.vector
                        lo_eng.tensor_tensor(
                            exp_lo[q_b][:, k_t // 2, k_t % 2, :], scr[:], dst_hi, SUB
                        )

                # rowsum over quantized exp (hi+lo) via DR ones-matmul,
                # then sbc = w * SA/SSN / rowsum broadcast over partitions.
                # Emission is deferred behind covering PE work so the ones-
                # matmuls never stall on the Pool exp_lo chain.
                def emit_rowsum(q_b):
                    # sum of exp_hi only: the exp_lo column-sum is a zero-mean
                    # ~0.06% correction, far below the error budget
                    ps_s = psp.tile([1, 512], f32, tag="acc", name="ps_s")
                    for j in range(4):
                        nc.tensor.matmul(
                            ps_s[:],
                            ones_t[:, :, 0:1],
                            exp_hi[q_b][:, j, :, :],
                            start=(j == 0),
                            stop=(j == 3),
                            perf_mode=DR,
                        )
                    rs = vecp.tile([1, 512], f32, tag="rs", name="rs")
                    nc.vector.reciprocal(rs[:], ps_s[:])
                    s_row = vecp.tile([1, 512], f32, tag="srow", name="s_row")
                    nc.vector.tensor_scalar_mul(
                        s_row[:], rs[:], wgt_sb[0:1, p : p + 1]
                    )
                    srow_d = dramp.tile([1, 512], f32, tag="srd", name="srow_d")
                    nc.sync.dma_start(srow_d[:], s_row[:])
                    sb_t = sbcp.tile([128, 512], f32, tag=f"sbc{q_b}", name="sb_t")
                    nc.sync.dma_start(sb_t[:], srow_d[0:1, :].partition_broadcast(128))
                    sbc[p][q_b] = sb_t

                emit_rowsum(0)

                if p == 0:
                    # path-1 HT prefetch slot behind path-0 compute
                    ht_hi[0] = load_pairs(htp, "hthi0", "ht_hi0", HT_hi_d[0])
                    ht_lo[0] = load_pairs(htp, "htlo0", "ht_lo0", HT_lo_d[0])

                # =====================================================
                # S3: AOS[d',q] = sum_k SN[k,d'] expT[k,q] ; scale + hi/lo
                # =====================================================
                aos_hi[p] = [
                    aosp.tile([128, 4, 2, 512], f8, tag=f"aoshi{p}{qb}",
                              name=f"aos_hi{p}")
                    for qb in range(2)
                ]
                aos_lo[p] = [
                    aosp.tile([128, 4, 2, 512], f8, tag=f"aoslo{p}{qb}",
                              name=f"aos_lo{p}")
                    for qb in range(2)
                ]
                s3_terms = [(sn_hi[p], exp_hi), (sn_lo[p], exp_hi), (sn_hi[p], exp_lo)]

                def emit_s3(q_b, dps):
                    for dp in dps:
                        ps = psp.tile([128, 512], f32, tag="acc", name="ps3")
                        k = 0
                        for (sn, et) in s3_terms:
                            for j in range(4):
                                nc.tensor.matmul(
                                    ps[:],
                                    sn[:, j, :, dp * 128 : (dp + 1) * 128],
                                    et[q_b][:, j, :, :],
                                    start=(k == 0),
                                    stop=(k == 11),
                                    perf_mode=DR,
                                )
                                k += 1
                        t32 = scrp.tile([128, 512], f32, tag="scr", name="t32")
                        nc.vector.tensor_tensor(t32[:], ps[:], sbc[p][q_b][:], MULT)
                        dst_hi = aos_hi[p][q_b][:, dp // 2, dp % 2, :]
                        nc.vector.tensor_copy(dst_hi, t32[:])
                        lo_eng = nc.gpsimd if dp % 2 == 0 else nc.vector
                        lo_eng.tensor_tensor(
                            aos_lo[p][q_b][:, dp // 2, dp % 2, :], t32[:], dst_hi, SUB
                        )

                emit_s3(0, range(0, 4))
                emit_rowsum(1)
                emit_s3(0, range(4, 8))
                emit_s3(1, range(0, 8))

                if p == 0:
                    ht_hi[1] = load_pairs(htp, "hthi1", "ht_hi1", HT_hi_d[1])
                    ht_lo[1] = load_pairs(htp, "htlo1", "ht_lo1", HT_lo_d[1])

            # =====================================================
            # S4: outT[o,q] = sum_p sum_d' HT_p[d',o] AOSs_p[d',q]
            # single accumulation over both paths (contraction 2048 x 3 terms)
            # =====================================================
            s4_terms = [
                (p, ht, at)
                for p in range(2)
                for (ht, at) in (
                    (ht_hi[p], aos_hi[p]),
                    (ht_lo[p], aos_hi[p]),
                    (ht_hi[p], aos_lo[p]),
                )
            ]

            def emit_s4_chain(o_t, q0, width):
                ps = psp.tile([128, width], f32, tag="acc", name="ps4")
                q_b = q0 // 512
                qsl = slice(q0 - q_b * 512, q0 - q_b * 512 + width)
                k = 0
                for (_, ht, at) in s4_terms:
                    for j in range(4):
                        nc.tensor.matmul(
                            ps[:],
                            ht[:, j, :, o_t * 128 : (o_t + 1) * 128],
                            at[q_b][:, j, :, qsl],
                            start=(k == 0),
                            stop=(k == 23),
                            perf_mode=DR,
                        )
                        k += 1
                osb = osbp.tile([128, width], f32, tag="osb", name="osb")
                nc.scalar.activation(
                    osb[:], ps[:], Identity,
                    bias=boe_t[:, o_t : o_t + 1],
                    scale=float(1.0 / (SH * SA)),
                )
                nc.sync.dma_start(
                    outT[o_t * 128 : (o_t + 1) * 128, q0 : q0 + width], osb[:]
                )

            for q_b in range(2):
                for o_t in range(8):
                    if q_b == 1 and o_t == 7:
                        # split the last tile so the final copy+DMA chain is
                        # short and pipelines behind the preceding matmuls
                        emit_s4_chain(o_t, 512, 256)
                        emit_s4_chain(o_t, 768, 128)
                        emit_s4_chain(o_t, 896, 128)
                    else:
                        emit_s4_chain(o_t, q_b * 512, 512)

    nc.compile()
    return nc


def _get_program():
    if "nc" not in _CACHE:
        _CACHE["nc"] = _build_program()
    return _CACHE["nc"]


def _host_gating(Q, Wq, bq, Wm1, bm1, Wm2, bm2):
    """Replicates the reference path-score MLP + top-k sparse weights."""
    Qm = Q.astype(np.float64).mean(axis=1)  # [B, D]
    pooled = Qm @ Wq.astype(np.float64).T + bq.astype(np.float64)
    h = np.maximum(pooled @ Wm1.astype(np.float64).T + bm1.astype(np.float64), 0.0)
    pl = h @ Wm2.astype(np.float64).T + bm2.astype(np.float64)  # [B, P]
    pl = pl - pl.max(axis=1, keepdims=True)
    e = np.exp(pl)
    scores = e / e.sum(axis=1, keepdims=True)
    idx = np.argsort(-scores, axis=1, kind="stable")[:, :TOP_K]  # [B, 2]
    w = np.take_along_axis(scores, idx, axis=1)
    wn = w / (w.sum(axis=1, keepdims=True) + 1e-8)
    return idx.astype(np.int64), wn.astype(np.float32)


def _q8_pair(x, scale):
    """x [1024, C] -> (hi, lo) fp8 pair tensors [128, 4, 2, C].

    Clips to the e4m3 max-normal (+-240) so extreme outliers saturate
    instead of becoming fp8 inf."""
    xs = (np.asarray(x, np.float32) * np.float32(scale)).astype(np.float32)
    xs = np.clip(xs, -240.0, 240.0)
    hi = xs.astype(E4)
    lo = np.clip(xs - hi.astype(np.float32), -240.0, 240.0).astype(E4)
    C = x.shape[1]

    def lay(a):
        return np.ascontiguousarray(
            a.reshape(4, 2, 128, C).transpose(2, 0, 1, 3)
        )

    return lay(hi), lay(lo)


def kernel(**inputs):
    from concourse.bass_utils import run_bass_kernel_spmd

    Q = np.asarray(inputs["Q"], dtype=np.float32)
    src = np.asarray(inputs["src"], dtype=np.float32)
    Wq = np.asarray(inputs["Wq"], dtype=np.float32)
    bq = np.asarray(inputs["bq"], dtype=np.float32)
    Wk = np.asarray(inputs["Wk"], dtype=np.float32)
    Wv = np.asarray(inputs["Wv"], dtype=np.float32)
    bv = np.asarray(inputs["bv"], dtype=np.float32)
    Wm1 = np.asarray(inputs["Wm1"], dtype=np.float32)
    bm1 = np.asarray(inputs["bm1"], dtype=np.float32)
    Wm2 = np.asarray(inputs["Wm2"], dtype=np.float32)
    bm2 = np.asarray(inputs["bm2"], dtype=np.float32)
    Wo = np.asarray(inputs["Wo"], dtype=np.float32)
    bo = np.asarray(inputs["bo"], dtype=np.float32)

    idx, wn = _host_gating(Q, Wq, bq, Wm1, bm1, Wm2, bm2)
    SCALE = 1.0 / float(np.sqrt(D))

    nc = _get_program()

    # host-folded weights, shared across cores (<=4 selected paths)
    sel = sorted(set(idx.flatten().tolist()))
    WqT = Wq.T
    G8 = {
        p: np.ascontiguousarray(np.concatenate(_q8_pair(WqT @ Wk[p], SG), axis=3))
        for p in sel
    }
    HT8 = {p: _q8_pair((Wo @ Wv[p]).T, SH) for p in sel}
    g2 = {p: Wk[p].T @ bq for p in sel}
    Wobv = {p: Wo @ bv[p] for p in sel}
    ones_pair = np.ones((128, 2, 16), dtype=E4)
    LN_SE = float(np.log(SE))

    in_maps = []
    for b in range(B):
        p0, p1 = int(idx[b, 0]), int(idx[b, 1])
        boe = bo + wn[b, 0] * Wobv[p0] + wn[b, 1] * Wobv[p1]
        qt_hi, qt_lo = _q8_pair(Q[b].T, SQ)
        m = {
            "QT_hi": qt_hi,
            "QT_lo": qt_lo,
            "boe": np.ascontiguousarray(boe.reshape(D, 1).astype(np.float32)),
            "wgt": np.ascontiguousarray(
                (wn[b] * (SA / SSN)).reshape(1, 2).astype(np.float32)
            ),
            "ones_pair": ones_pair,
        }
        for i, p in enumerate((p0, p1)):
            S = src[p, b]
            m[f"G{i}_pair"] = G8[p]
            m[f"ST{i}_hi"], m[f"ST{i}_lo"] = _q8_pair(S.T, ST_)
            m[f"SN{i}_hi"], m[f"SN{i}_lo"] = _q8_pair(S, SSN)
            m[f"HT{i}_hi"], m[f"HT{i}_lo"] = HT8[p]
            m[f"vb{i}"] = np.ascontiguousarray(
                ((S @ g2[p]) * SCALE + LN_SE).reshape(LK, 1).astype(np.float32)
            )
        in_maps.append(m)

    res = run_bass_kernel_spmd(nc, in_maps, core_ids=list(range(N_CORES)))
    out = np.stack([res.results[b]["outT"].T for b in range(B)], axis=0)
    return np.ascontiguousarray(out).astype(np.float32)


# revision 21
# speedup vs baseline: 1.3993x; 1.0199x over previous
"""Trainium2 Bass kernel for DynamicPathCrossAttention.

Sharding: batch-parallel — core b computes batch element b end-to-end. The
path-gating MLP runs on the host; each core computes cross-attention for its
batch element's TOP_K=2 selected paths only.

Weight folding (host, shared across cores): the chain is linear around the
softmax, so adjacent projection pairs collapse:
  logits = Q Wq^T Wk S^T          -> G_p = Wq^T @ Wk_p     (logits = Q G S^T)
  out    = attn S Wv^T Wo^T (...) -> H_p = Wo @ Wv_p       (out = attn S H^T)
Bias algebra: per-q logit terms cancel in softmax; the per-k term ships as an
exp() bias column vb; bv folds into an effective output bias boe.

All matmuls run as fp8e4 (e4m3) DoubleRow with hi/lo error compensation:
every operand X is split on host (or on device for intermediates) into
  X_hi = fp8(X*s),  X_lo = fp8(X*s - X_hi)
and each contraction A@B is computed as A_hi@B_hi + A_lo@B_hi + A_hi@B_lo
(the lo*lo term ~eps^2 is dropped), keeping rel err ~2e-3 at 2x the f32r
matmul rate. DoubleRow packs two 128-row contraction planes per matmul:
operands are laid out [128 part, 2 planes, cols]; logical contraction index
d = 256*j + 128*i + p for pair-tile j, plane i, partition p.

Per-core pipeline (scales are powers of 2, folded into ACT scale factors):
  S1: TMP[d',q] = G^T QT      (G,QT host fp8 pairs; TMP hi/lo via ACT+DVE)
  S2: logitsT[k,q] = ST^T TMP ; expT = exp(.*SCALE + vb + ln sE) (ACT, f32)
      exp hi cast (ACT) + lo residual (Pool tensor_tensor sub)
  rowsum = DR ones-matmul over exp_hi pairs (the exp_lo column-sum is a
      zero-mean ~0.1% correction, below the error budget); sbc = w/rowsum
      broadcast (per-q) via DRAM-bounce partition_broadcast
  S3: AOS[d',q] = SN^T expT ; AOSs = AOS * sbc (DVE), hi cast (DVE) +
      lo residual (Pool)
  S4 (both paths in one accumulation, contraction 2048):
      outT[o,q] = sum_p HT_p^T AOSs_p ; final ACT copy applies 2^-19 + boe.

A memset-fed warmup matmul chain keeps the PE busy from t~1us so the
p-state ramp completes before the first real matmul (DMA-latency window).
"""

import numpy as np
import ml_dtypes

D = 1024
P = 4
TOP_K = 2
B = 8
LQ = 1024
LK = 1024
N_CORES = 8

E4 = ml_dtypes.float8_e4m3

# power-of-2 quantization scales
SG = 256.0    # G
SQ = 8.0      # QT
ST_ = 8.0     # S^T (stage2 stationary)
ST__ = 16.0   # TMP
SE = 1.0      # expT (ln SE folds into the exp bias)
SSN = 8.0     # SN (stage3 stationary)
SA = 128.0    # AOSs
SH = 256.0    # HT

# compensation flags (both sides of every stage compensated by default)
COMP_QT = True
COMP_TMP = True
N_QTC = 3     # QT-compensation pairs (of 4)

N_WARMUP = 50

_CACHE = {}


def _build_program():
    import concourse.mybir as mybir
    import concourse.tile as tile
    from concourse import bacc

    f32 = mybir.dt.float32
    f8 = mybir.dt.float8e4
    DR = mybir.MatmulPerfMode.DoubleRow
    Exp = mybir.ActivationFunctionType.Exp
    Identity = mybir.ActivationFunctionType.Identity
    MULT = mybir.AluOpType.mult
    SUB = mybir.AluOpType.subtract

    SCALE = 1.0 / float(np.sqrt(D))

    nc = bacc.Bacc(
        "TRN2", target_bir_lowering=False, debug=False, enable_asserts=False
    )

    def din(name, shape, dt=f8):
        return nc.dram_tensor(name, shape, dt, kind="ExternalInput").ap()

    # host-prepped fp8 pair tensors: [128 part, 4 pair, 2 plane, 1024 cols]
    QT_hi_d = din("QT_hi", [128, 4, 2, LQ])
    QT_lo_d = din("QT_lo", [128, 4, 2, LQ])
    G_pair_d = [din(f"G{p}_pair", [128, 4, 2, 2 * D]) for p in range(2)]
    ST_hi_d = [din(f"ST{p}_hi", [128, 4, 2, LK]) for p in range(2)]
    ST_lo_d = [din(f"ST{p}_lo", [128, 4, 2, LK]) for p in range(2)]
    SN_hi_d = [din(f"SN{p}_hi", [128, 4, 2, D]) for p in range(2)]
    SN_lo_d = [din(f"SN{p}_lo", [128, 4, 2, D]) for p in range(2)]
    HT_hi_d = [din(f"HT{p}_hi", [128, 4, 2, D]) for p in range(2)]
    HT_lo_d = [din(f"HT{p}_lo", [128, 4, 2, D]) for p in range(2)]
    vb_d = [din(f"vb{p}", [LK, 1], f32) for p in range(2)]
    boe_c = din("boe", [D, 1], f32)
    wgt = din("wgt", [1, 2], f32)  # w_p * SA / SSN
    ones_d = din("ones_pair", [128, 2, 16])
    outT = nc.dram_tensor("outT", [D, LQ], f32, kind="ExternalOutput").ap()

    with tile.TileContext(nc) as tc:
        import contextlib

        with contextlib.ExitStack() as ctx:
            const = ctx.enter_context(tc.tile_pool(name="const", bufs=1))
            warmp = ctx.enter_context(tc.tile_pool(name="warmp", bufs=1))
            qtp = ctx.enter_context(tc.tile_pool(name="qtp", bufs=1))
            gp = ctx.enter_context(tc.tile_pool(name="gp", bufs=1))
            stp = ctx.enter_context(tc.tile_pool(name="stp", bufs=1))
            snp = ctx.enter_context(tc.tile_pool(name="snp", bufs=1))
            htp = ctx.enter_context(tc.tile_pool(name="htp", bufs=1))
            tmpp = ctx.enter_context(tc.tile_pool(name="tmpp", bufs=1))
            expp = ctx.enter_context(tc.tile_pool(name="expp", bufs=1))
            aosp = ctx.enter_context(tc.tile_pool(name="aosp", bufs=1))
            scrp = ctx.enter_context(tc.tile_pool(name="scrp", bufs=3))
            vecp = ctx.enter_context(tc.tile_pool(name="vecp", bufs=2))
            sbcp = ctx.enter_context(tc.tile_pool(name="sbcp", bufs=2))
            osbp = ctx.enter_context(tc.tile_pool(name="osbp", bufs=4))
            psp = ctx.enter_context(tc.tile_pool(name="psp", bufs=8, space="PSUM"))
            dramp = ctx.enter_context(tc.tile_pool(name="dramp", bufs=2, space="DRAM"))

            # ---- warmup: keep PE busy through the p-state ramp while the
            # first input DMAs are in flight
            warm = warmp.tile([128, 2, 128], f8)
            nc.vector.memset(warm[:], 0)
            wps_t = psp.tile([16, 128], f32, tag="acc", name="wps_t")
            for _ in range(N_WARMUP):
                nc.tensor.matmul(
                    wps_t[:], warm[:, :, 0:16], warm[:], start=True, stop=True,
                    perf_mode=DR,
                )

            # ---- input DMAs, ordered to match stage-1's j-outer consumption:
            # per pair j deliver (QT_hi half, G_hi, G_lo) so hi+lo terms of
            # pair j can run while pair j+1 streams
            qt_hi = qtp.tile([128, 4, 2, LQ], f8)
            qt_lo = qtp.tile([128, 4, 2, LQ], f8)
            g_pair = [None, None]
            g_pair[0] = gp.tile([128, 4, 2, 2 * D], f8, tag="gpair", name="g_pair0")
            for j in range(4):
                nc.sync.dma_start(qt_hi[:, j, :, 0:512], QT_hi_d[:, j, :, 0:512])
                nc.sync.dma_start(g_pair[0][:, j], G_pair_d[0][:, j])
            if COMP_QT:
                nc.sync.dma_start(
                    qt_lo[:, 0:N_QTC, :, 0:512], QT_lo_d[:, 0:N_QTC, :, 0:512]
                )
            nc.sync.dma_start(qt_hi[:, :, :, 512:1024], QT_hi_d[:, :, :, 512:1024])
            if COMP_QT:
                nc.sync.dma_start(
                    qt_lo[:, 0:N_QTC, :, 512:1024],
                    QT_lo_d[:, 0:N_QTC, :, 512:1024],
                )

            # vb before the stage-2 operands (needed at the first exp)
            vb_t = [const.tile([128, 8], f32, name=f"vb_t{p}") for p in range(2)]
            for pp in range(2):
                nc.sync.dma_start(
                    vb_t[pp][:], vb_d[pp].rearrange("(t p) o -> p (t o)", p=128)
                )

            st_hi = [None, None]
            st_lo = [None, None]
            sn_hi = [None, None]
            sn_lo = [None, None]
            ht_hi = [None, None]
            ht_lo = [None, None]

            def load_pairs(pool, tag, name, dram):
                t = pool.tile([128, 4, 2, D], f8, tag=tag, name=name)
                nc.sync.dma_start(t[:], dram[:])
                return t

            # stage-2/3 operands for path 0 follow behind the stage-1 set
            st_hi[0] = load_pairs(stp, "sthi", "st_hi0", ST_hi_d[0])
            st_lo[0] = load_pairs(stp, "stlo", "st_lo0", ST_lo_d[0])

            # remaining small constants (rowsum / sbc / S4)
            ones_t = const.tile([128, 2, 16], f8)
            nc.sync.dma_start(ones_t[:], ones_d[:])
            wgt_sb = const.tile([1, 2], f32)
            nc.sync.dma_start(wgt_sb[:], wgt[:])
            boe_t = const.tile([128, 8], f32)
            nc.sync.dma_start(boe_t[:], boe_c.rearrange("(t p) o -> p (t o)", p=128))

            sn_hi[0] = load_pairs(snp, "snhi", "sn_hi0", SN_hi_d[0])
            sn_lo[0] = load_pairs(snp, "snlo", "sn_lo0", SN_lo_d[0])

            aos_hi = [None, None]
            aos_lo = [None, None]
            sbc = [[None, None], [None, None]]

            for p in range(2):
                if p == 1:
                    # path-1 operands (slots freed by path 0 reuse via tags)
                    g_pair[1] = gp.tile(
                        [128, 4, 2, 2 * D], f8, tag="gpair", name="g_pair1"
                    )
                    nc.sync.dma_start(g_pair[1][:], G_pair_d[1][:])
                    st_hi[1] = load_pairs(stp, "sthi", "st_hi1", ST_hi_d[1])
                    st_lo[1] = load_pairs(stp, "stlo", "st_lo1", ST_lo_d[1])
                    sn_hi[1] = load_pairs(snp, "snhi", "sn_hi1", SN_hi_d[1])
                    sn_lo[1] = load_pairs(snp, "snlo", "sn_lo1", SN_lo_d[1])

                # =====================================================
                # S1: TMP[d',q] = sum_d G[d,d'] QT[d,q]   (DR pairs over d)
                # =====================================================
                tmp_hi = [
                    tmpp.tile([128, 4, 2, 512], f8, tag=f"tmphi{qb}", name="tmp_hi")
                    for qb in range(2)
                ]
                tmp_lo = [
                    tmpp.tile([128, 4, 2, 512], f8, tag=f"tmplo{qb}", name="tmp_lo")
                    for qb in range(2)
                ]
                # (j, term) consumption order matches the DMA delivery order:
                # pair j's hi+lo G terms run back-to-back, QT_lo term last
                s1_sched = [(j, t) for j in range(4) for t in range(2)]
                if COMP_QT:
                    # compensate the QT quantization on 3/4 of the contraction
                    # (pairs j<3): the residual quarter contributes ~0.5% final
                    # error, well inside the budget, and saves a DR pass chunk
                    s1_sched += [(j, 2) for j in range(N_QTC)]
                s1_ops = [(0, qt_hi), (D, qt_hi), (0, qt_lo)]
                n_mm = len(s1_sched)
                def s1_cast(ps, dp, qb):
                    dst_hi = tmp_hi[qb][:, dp // 2, dp % 2, :]
                    nc.scalar.activation(
                        dst_hi, ps[:], Identity, scale=float(ST__ / (SG * SQ))
                    )
                    if COMP_TMP:
                        nc.vector.scalar_tensor_tensor(
                            tmp_lo[qb][:, dp // 2, dp % 2, :],
                            ps[:],
                            float(ST__ / (SG * SQ)),
                            dst_hi,
                            MULT,
                            SUB,
                        )

                if p == 0:
                    # (j, term)-outer in 4-dp halves — consumption tracks DMA
                    # delivery (lo terms last); each half's PSUM banks drain
                    # while the next half runs
                    for q_b0 in range(2):
                        qs = slice(q_b0 * 512, (q_b0 + 1) * 512)
                        for half in range(2):
                            dps = range(half * 4, half * 4 + 4)
                            ps_t = {
                                dp: psp.tile(
                                    [128, 512], f32, tag="acc", name="ps1"
                                )
                                for dp in dps
                            }
                            for k, (j, t) in enumerate(s1_sched):
                                off, qt = s1_ops[t]
                                for dp in dps:
                                    nc.tensor.matmul(
                                        ps_t[dp][:],
                                        g_pair[p][
                                            :, j, :,
                                            off + dp * 128 : off + (dp + 1) * 128,
                                        ],
                                        qt[:, j, :, qs],
                                        start=(k == 0),
                                        stop=(k == n_mm - 1),
                                        perf_mode=DR,
                                    )
                            for dp in dps:
                                s1_cast(ps_t[dp], dp, q_b0)
                    qb1_list = []
                else:
                    qb1_list = [0, 1]

                # dp-outer — staggers PSUM bank release for pipelining
                for q_b1 in qb1_list:
                  qs = slice(q_b1 * 512, (q_b1 + 1) * 512)
                  for dp in range(8):
                    ps = psp.tile([128, 512], f32, tag="acc", name="ps1b")
                    for k, (j, t) in enumerate(s1_sched):
                        off, qt = s1_ops[t]
                        nc.tensor.matmul(
                            ps[:],
                            g_pair[p][
                                :, j, :, off + dp * 128 : off + (dp + 1) * 128
                            ],
                            qt[:, j, :, qs],
                            start=(k == 0),
                            stop=(k == n_mm - 1),
                            perf_mode=DR,
                        )
                    s1_cast(ps, dp, q_b1)
                del qb1_list

                # =====================================================
                # S2: logitsT[k,q] = sum_d' ST[d',k] TMP[d',q] ; exp + hi/lo
                # =====================================================
                exp_hi = [
                    expp.tile([128, 4, 2, 512], f8, tag=f"exphi{qb}", name="exp_hi")
                    for qb in range(2)
                ]
                exp_lo = [
                    expp.tile([128, 4, 2, 512], f8, tag=f"explo{qb}", name="exp_lo")
                    for qb in range(2)
                ]
                s2_terms = [(st_hi[p], tmp_hi), (st_lo[p], tmp_hi)]
                if COMP_TMP:
                    s2_terms.append((st_hi[p], tmp_lo))
                for q_b in range(2):
                    for k_t in range(8):
                        ps = psp.tile([128, 512], f32, tag="acc", name="ps2")
                        n_mm = len(s2_terms) * 4
                        k = 0
                        for (st_, tm) in s2_terms:
                            for j in range(4):
                                nc.tensor.matmul(
                                    ps[:],
                                    st_[:, j, :, k_t * 128 : (k_t + 1) * 128],
                                    tm[q_b][:, j, :, :],
                                    start=(k == 0),
                                    stop=(k == n_mm - 1),
                                    perf_mode=DR,
                                )
                                k += 1
                        scr = scrp.tile([128, 512], f32, tag="scr", name="scr")
                        nc.scalar.activation(
                            scr[:], ps[:], Exp,
                            bias=vb_t[p][:, k_t : k_t + 1],
                            scale=float(SCALE / (ST_ * ST__)),
                        )
                        dst_hi = exp_hi[q_b][:, k_t // 2, k_t % 2, :]
                        nc.scalar.activation(dst_hi, scr[:], Identity)
                        lo_eng = nc.gpsimd if k_t % 2 == 0 else nc.vector
                        lo_eng.tensor_tensor(
                            exp_lo[q_b][:, k_t // 2, k_t % 2, :], scr[:], dst_hi, SUB
                        )

                # rowsum over quantized exp (hi+lo) via DR ones-matmul,
                # then sbc = w * SA/SSN / rowsum broadcast over partitions.
                # Emission is deferred behind covering PE work so the ones-
                # matmuls never stall on the Pool exp_lo chain.
                def emit_rowsum(q_b):
                    # sum of exp_hi only: the exp_lo column-sum is a zero-mean
                    # ~0.06% correction, far below the error budget
                    ps_s = psp.tile([1, 512], f32, tag="acc", name="ps_s")
                    for j in range(4):
                        nc.tensor.matmul(
                            ps_s[:],
                            ones_t[:, :, 0:1],
                            exp_hi[q_b][:, j, :, :],
                            start=(j == 0),
                            stop=(j == 3),
                            perf_mode=DR,
                        )
                    rs = vecp.tile([1, 512], f32, tag="rs", name="rs")
                    nc.vector.reciprocal(rs[:], ps_s[:])
                    s_row = vecp.tile([1, 512], f32, tag="srow", name="s_row")
                    nc.vector.tensor_scalar_mul(
                        s_row[:], rs[:], wgt_sb[0:1, p : p + 1]
                    )
                    srow_d = dramp.tile([1, 512], f32, tag="srd", name="srow_d")
                    nc.sync.dma_start(srow_d[:], s_row[:])
                    sb_t = sbcp.tile([128, 512], f32, tag=f"sbc{q_b}", name="sb_t")
                    nc.sync.dma_start(sb_t[:], srow_d[0:1, :].partition_broadcast(128))
                    sbc[p][q_b] = sb_t

                emit_rowsum(0)

                if p == 0:
                    # path-1 HT prefetch slot behind path-0 compute
                    ht_hi[0] = load_pairs(htp, "hthi0", "ht_hi0", HT_hi_d[0])
                    ht_lo[0] = load_pairs(htp, "htlo0", "ht_lo0", HT_lo_d[0])

                # =====================================================
                # S3: AOS[d',q] = sum_k SN[k,d'] expT[k,q] ; scale + hi/lo
                # =====================================================
                aos_hi[p] = [
                    aosp.tile([128, 4, 2, 512], f8, tag=f"aoshi{p}{qb}",
                              name=f"aos_hi{p}")
                    for qb in range(2)
                ]
                aos_lo[p] = [
                    aosp.tile([128, 4, 2, 512], f8, tag=f"aoslo{p}{qb}",
                              name=f"aos_lo{p}")
                    for qb in range(2)
                ]
                s3_terms = [(sn_hi[p], exp_hi), (sn_lo[p], exp_hi), (sn_hi[p], exp_lo)]

                def emit_s3(q_b, dps):
                    for dp in dps:
                        ps = psp.tile([128, 512], f32, tag="acc", name="ps3")
                        k = 0
                        for (sn, et) in s3_terms:
                            for j in range(4):
                                nc.tensor.matmul(
                                    ps[:],
                                    sn[:, j, :, dp * 128 : (dp + 1) * 128],
                                    et[q_b][:, j, :, :],
                                    start=(k == 0),
                                    stop=(k == 11),
                                    perf_mode=DR,
                                )
                                k += 1
                        t32 = scrp.tile([128, 512], f32, tag="scr", name="t32")
                        nc.vector.tensor_tensor(t32[:], ps[:], sbc[p][q_b][:], MULT)
                        dst_hi = aos_hi[p][q_b][:, dp // 2, dp % 2, :]
                        nc.vector.tensor_copy(dst_hi, t32[:])
                        lo_eng = nc.gpsimd if dp % 2 == 0 else nc.vector
                        lo_eng.tensor_tensor(
                            aos_lo[p][q_b][:, dp // 2, dp % 2, :], t32[:], dst_hi, SUB
                        )

                emit_s3(0, range(0, 4))
                emit_rowsum(1)
                emit_s3(0, range(4, 8))
                emit_s3(1, range(0, 8))

                if p == 0:
                    ht_hi[1] = load_pairs(htp, "hthi1", "ht_hi1", HT_hi_d[1])
                    ht_lo[1] = load_pairs(htp, "htlo1", "ht_lo1", HT_lo_d[1])

            # =====================================================
            # S4: outT[o,q] = sum_p sum_d' HT_p[d',o] AOSs_p[d',q]
            # single accumulation over both paths (contraction 2048 x 3 terms)
            # =====================================================
            s4_terms = [
                (p, ht, at)
                for p in range(2)
                for (ht, at) in (
                    (ht_hi[p], aos_hi[p]),
                    (ht_lo[p], aos_hi[p]),
                    (ht_hi[p], aos_lo[p]),
                )
            ]

            def emit_s4_chain(o_t, q0, width):
                ps = psp.tile([128, width], f32, tag="acc", name="ps4")
                q_b = q0 // 512
                qsl = slice(q0 - q_b * 512, q0 - q_b * 512 + width)
                k = 0
                for (_, ht, at) in s4_terms:
                    for j in range(4):
                        nc.tensor.matmul(
                            ps[:],
                            ht[:, j, :, o_t * 128 : (o_t + 1) * 128],
                            at[q_b][:, j, :, qsl],
                            start=(k == 0),
                            stop=(k == 23),
                            perf_mode=DR,
                        )
                        k += 1
                osb = osbp.tile([128, width], f32, tag="osb", name="osb")
                nc.scalar.activation(
                    osb[:], ps[:], Identity,
                    bias=boe_t[:, o_t : o_t + 1],
                    scale=float(1.0 / (SH * SA)),
                )
                nc.sync.dma_start(
                    outT[o_t * 128 : (o_t + 1) * 128, q0 : q0 + width], osb[:]
                )

            for q_b in range(2):
                for o_t in range(8):
                    if q_b == 1 and o_t == 7:
                        # split the last tile so the final copy+DMA chain is
                        # short and pipelines behind the preceding matmuls
                        emit_s4_chain(o_t, 512, 256)
                        emit_s4_chain(o_t, 768, 128)
                        emit_s4_chain(o_t, 896, 128)
                    else:
                        emit_s4_chain(o_t, q_b * 512, 512)

    nc.compile()
    return nc


def _get_program():
    if "nc" not in _CACHE:
        _CACHE["nc"] = _build_program()
    return _CACHE["nc"]


def _host_gating(Q, Wq, bq, Wm1, bm1, Wm2, bm2):
    """Replicates the reference path-score MLP + top-k sparse weights."""
    Qm = Q.astype(np.float64).mean(axis=1)  # [B, D]
    pooled = Qm @ Wq.astype(np.float64).T + bq.astype(np.float64)
    h = np.maximum(pooled @ Wm1.astype(np.float64).T + bm1.astype(np.float64), 0.0)
    pl = h @ Wm2.astype(np.float64).T + bm2.astype(np.float64)  # [B, P]
    pl = pl - pl.max(axis=1, keepdims=True)
    e = np.exp(pl)
    scores = e / e.sum(axis=1, keepdims=True)
    idx = np.argsort(-scores, axis=1, kind="stable")[:, :TOP_K]  # [B, 2]
    w = np.take_along_axis(scores, idx, axis=1)
    wn = w / (w.sum(axis=1, keepdims=True) + 1e-8)
    return idx.astype(np.int64), wn.astype(np.float32)


def _q8_pair(x, scale):
    """x [1024, C] -> (hi, lo) fp8 pair tensors [128, 4, 2, C].

    Clips to the e4m3 max-normal (+-240) so extreme outliers saturate
    instead of becoming fp8 inf."""
    xs = (np.asarray(x, np.float32) * np.float32(scale)).astype(np.float32)
    xs = np.clip(xs, -240.0, 240.0)
    hi = xs.astype(E4)
    lo = np.clip(xs - hi.astype(np.float32), -240.0, 240.0).astype(E4)
    C = x.shape[1]

    def lay(a):
        return np.ascontiguousarray(
            a.reshape(4, 2, 128, C).transpose(2, 0, 1, 3)
        )

    return lay(hi), lay(lo)


def kernel(**inputs):
    from concourse.bass_utils import run_bass_kernel_spmd

    Q = np.asarray(inputs["Q"], dtype=np.float32)
    src = np.asarray(inputs["src"], dtype=np.float32)
    Wq = np.asarray(inputs["Wq"], dtype=np.float32)
    bq = np.asarray(inputs["bq"], dtype=np.float32)
    Wk = np.asarray(inputs["Wk"], dtype=np.float32)
    Wv = np.asarray(inputs["Wv"], dtype=np.float32)
    bv = np.asarray(inputs["bv"], dtype=np.float32)
    Wm1 = np.asarray(inputs["Wm1"], dtype=np.float32)
    bm1 = np.asarray(inputs["bm1"], dtype=np.float32)
    Wm2 = np.asarray(inputs["Wm2"], dtype=np.float32)
    bm2 = np.asarray(inputs["bm2"], dtype=np.float32)
    Wo = np.asarray(inputs["Wo"], dtype=np.float32)
    bo = np.asarray(inputs["bo"], dtype=np.float32)

    idx, wn = _host_gating(Q, Wq, bq, Wm1, bm1, Wm2, bm2)
    SCALE = 1.0 / float(np.sqrt(D))

    nc = _get_program()

    # host-folded weights, shared across cores (<=4 selected paths)
    sel = sorted(set(idx.flatten().tolist()))
    WqT = Wq.T
    G8 = {
        p: np.ascontiguousarray(np.concatenate(_q8_pair(WqT @ Wk[p], SG), axis=3))
        for p in sel
    }
    HT8 = {p: _q8_pair((Wo @ Wv[p]).T, SH) for p in sel}
    g2 = {p: Wk[p].T @ bq for p in sel}
    Wobv = {p: Wo @ bv[p] for p in sel}
    ones_pair = np.ones((128, 2, 16), dtype=E4)
    LN_SE = float(np.log(SE))

    in_maps = []
    for b in range(B):
        p0, p1 = int(idx[b, 0]), int(idx[b, 1])
        boe = bo + wn[b, 0] * Wobv[p0] + wn[b, 1] * Wobv[p1]
        qt_hi, qt_lo = _q8_pair(Q[b].T, SQ)
        m = {
            "QT_hi": qt_hi,
            "QT_lo": qt_lo,
            "boe": np.ascontiguousarray(boe.reshape(D, 1).astype(np.float32)),
            "wgt": np.ascontiguousarray(
                (wn[b] * (SA / SSN)).reshape(1, 2).astype(np.float32)
            ),
            "ones_pair": ones_pair,
        }
        for i, p in enumerate((p0, p1)):
            S = src[p, b]
            m[f"G{i}_pair"] = G8[p]
            m[f"ST{i}_hi"], m[f"ST{i}_lo"] = _q8_pair(S.T, ST_)
            m[f"SN{i}_hi"], m[f"SN{i}_lo"] = _q8_pair(S, SSN)
            m[f"HT{i}_hi"], m[f"HT{i}_lo"] = HT8[p]
            m[f"vb{i}"] = np.ascontiguousarray(
                ((S @ g2[p]) * SCALE + LN_SE).reshape(LK, 1).astype(np.float32)
            )
        in_maps.append(m)

    res = run_bass_kernel_spmd(nc, in_maps, core_ids=list(range(N_CORES)))
    out = np.stack([res.results[b]["outT"].T for b in range(B)], axis=0)
    return np.ascontiguousarray(out).astype(np.float32)


# revision 22
# speedup vs baseline: 1.4272x; 1.0200x over previous
"""Trainium2 Bass kernel for DynamicPathCrossAttention.

Sharding: batch-parallel — core b computes batch element b end-to-end. The
path-gating MLP runs on the host; each core computes cross-attention for its
batch element's TOP_K=2 selected paths only.

Weight folding (host, shared across cores): the chain is linear around the
softmax, so adjacent projection pairs collapse:
  logits = Q Wq^T Wk S^T          -> G_p = Wq^T @ Wk_p     (logits = Q G S^T)
  out    = attn S Wv^T Wo^T (...) -> H_p = Wo @ Wv_p       (out = attn S H^T)
Bias algebra: per-q logit terms cancel in softmax; the per-k term ships as an
exp() bias column vb; bv folds into an effective output bias boe.

All matmuls run as fp8e4 (e4m3) DoubleRow with hi/lo error compensation:
every operand X is split on host (or on device for intermediates) into
  X_hi = fp8(X*s),  X_lo = fp8(X*s - X_hi)
and each contraction A@B is computed as A_hi@B_hi + A_lo@B_hi + A_hi@B_lo
(the lo*lo term ~eps^2 is dropped), keeping rel err ~2e-3 at 2x the f32r
matmul rate. DoubleRow packs two 128-row contraction planes per matmul:
operands are laid out [128 part, 2 planes, cols]; logical contraction index
d = 256*j + 128*i + p for pair-tile j, plane i, partition p.

Per-core pipeline (scales are powers of 2, folded into ACT scale factors):
  S1: TMP[d',q] = G^T QT      (G,QT host fp8 pairs; TMP hi/lo via ACT+DVE)
  S2: logitsT[k,q] = ST^T TMP ; expT = exp(.*SCALE + vb + ln sE) (ACT, f32)
      exp hi cast (ACT) + lo residual (Pool tensor_tensor sub)
  rowsum = DR ones-matmul over exp_hi pairs (the exp_lo column-sum is a
      zero-mean ~0.1% correction, below the error budget); sbc = w/rowsum
      broadcast (per-q) via DRAM-bounce partition_broadcast
  S3: AOS[d',q] = SN^T expT ; AOSs = AOS * sbc (DVE), hi cast (DVE) +
      lo residual (Pool)
  S4 (both paths in one accumulation, contraction 2048):
      outT[o,q] = sum_p HT_p^T AOSs_p ; final ACT copy applies 2^-19 + boe.

A memset-fed warmup matmul chain keeps the PE busy from t~1us so the
p-state ramp completes before the first real matmul (DMA-latency window).
"""

import numpy as np
import ml_dtypes

D = 1024
P = 4
TOP_K = 2
B = 8
LQ = 1024
LK = 1024
N_CORES = 8

E4 = ml_dtypes.float8_e4m3

# power-of-2 quantization scales
SG = 256.0    # G
SQ = 8.0      # QT
ST_ = 8.0     # S^T (stage2 stationary)
ST__ = 16.0   # TMP
SE = 1.0      # expT (ln SE folds into the exp bias)
SSN = 8.0     # SN (stage3 stationary)
SA = 128.0    # AOSs
SH = 256.0    # HT

# compensation flags (both sides of every stage compensated by default)
COMP_QT = True
COMP_TMP = True
N_QTC = 2     # QT-compensation pairs (of 4)

N_WARMUP = 50

_CACHE = {}


def _build_program():
    import concourse.mybir as mybir
    import concourse.tile as tile
    from concourse import bacc

    f32 = mybir.dt.float32
    f8 = mybir.dt.float8e4
    DR = mybir.MatmulPerfMode.DoubleRow
    Exp = mybir.ActivationFunctionType.Exp
    Identity = mybir.ActivationFunctionType.Identity
    MULT = mybir.AluOpType.mult
    SUB = mybir.AluOpType.subtract

    SCALE = 1.0 / float(np.sqrt(D))

    nc = bacc.Bacc(
        "TRN2", target_bir_lowering=False, debug=False, enable_asserts=False
    )

    def din(name, shape, dt=f8):
        return nc.dram_tensor(name, shape, dt, kind="ExternalInput").ap()

    # host-prepped fp8 pair tensors: [128 part, 4 pair, 2 plane, 1024 cols]
    QT_hi_d = din("QT_hi", [128, 4, 2, LQ])
    QT_lo_d = din("QT_lo", [128, 4, 2, LQ])
    G_pair_d = [din(f"G{p}_pair", [128, 4, 2, 2 * D]) for p in range(2)]
    ST_hi_d = [din(f"ST{p}_hi", [128, 4, 2, LK]) for p in range(2)]
    ST_lo_d = [din(f"ST{p}_lo", [128, 4, 2, LK]) for p in range(2)]
    SN_hi_d = [din(f"SN{p}_hi", [128, 4, 2, D]) for p in range(2)]
    SN_lo_d = [din(f"SN{p}_lo", [128, 4, 2, D]) for p in range(2)]
    HT_hi_d = [din(f"HT{p}_hi", [128, 4, 2, D]) for p in range(2)]
    HT_lo_d = [din(f"HT{p}_lo", [128, 4, 2, D]) for p in range(2)]
    vb_d = [din(f"vb{p}", [LK, 1], f32) for p in range(2)]
    boe_c = din("boe", [D, 1], f32)
    wgt = din("wgt", [1, 2], f32)  # w_p * SA / SSN
    ones_d = din("ones_pair", [128, 2, 16])
    outT = nc.dram_tensor("outT", [D, LQ], f32, kind="ExternalOutput").ap()

    with tile.TileContext(nc) as tc:
        import contextlib

        with contextlib.ExitStack() as ctx:
            const = ctx.enter_context(tc.tile_pool(name="const", bufs=1))
            warmp = ctx.enter_context(tc.tile_pool(name="warmp", bufs=1))
            qtp = ctx.enter_context(tc.tile_pool(name="qtp", bufs=1))
            gp = ctx.enter_context(tc.tile_pool(name="gp", bufs=1))
            stp = ctx.enter_context(tc.tile_pool(name="stp", bufs=1))
            snp = ctx.enter_context(tc.tile_pool(name="snp", bufs=1))
            htp = ctx.enter_context(tc.tile_pool(name="htp", bufs=1))
            tmpp = ctx.enter_context(tc.tile_pool(name="tmpp", bufs=1))
            expp = ctx.enter_context(tc.tile_pool(name="expp", bufs=1))
            aosp = ctx.enter_context(tc.tile_pool(name="aosp", bufs=1))
            scrp = ctx.enter_context(tc.tile_pool(name="scrp", bufs=3))
            vecp = ctx.enter_context(tc.tile_pool(name="vecp", bufs=2))
            sbcp = ctx.enter_context(tc.tile_pool(name="sbcp", bufs=2))
            osbp = ctx.enter_context(tc.tile_pool(name="osbp", bufs=4))
            psp = ctx.enter_context(tc.tile_pool(name="psp", bufs=8, space="PSUM"))
            dramp = ctx.enter_context(tc.tile_pool(name="dramp", bufs=2, space="DRAM"))

            # ---- warmup: keep PE busy through the p-state ramp while the
            # first input DMAs are in flight
            warm = warmp.tile([128, 2, 128], f8)
            nc.vector.memset(warm[:], 0)
            wps_t = psp.tile([16, 128], f32, tag="acc", name="wps_t")
            for _ in range(N_WARMUP):
                nc.tensor.matmul(
                    wps_t[:], warm[:, :, 0:16], warm[:], start=True, stop=True,
                    perf_mode=DR,
                )

            # ---- input DMAs, ordered to match stage-1's j-outer consumption:
            # per pair j deliver (QT_hi half, G_hi, G_lo) so hi+lo terms of
            # pair j can run while pair j+1 streams
            qt_hi = qtp.tile([128, 4, 2, LQ], f8)
            qt_lo = qtp.tile([128, 4, 2, LQ], f8)
            g_pair = [None, None]
            g_pair[0] = gp.tile([128, 4, 2, 2 * D], f8, tag="gpair", name="g_pair0")
            for j in range(4):
                nc.sync.dma_start(qt_hi[:, j, :, 0:512], QT_hi_d[:, j, :, 0:512])
                nc.sync.dma_start(g_pair[0][:, j], G_pair_d[0][:, j])
            if COMP_QT:
                nc.sync.dma_start(
                    qt_lo[:, 0:N_QTC, :, 0:512], QT_lo_d[:, 0:N_QTC, :, 0:512]
                )
            nc.sync.dma_start(qt_hi[:, :, :, 512:1024], QT_hi_d[:, :, :, 512:1024])
            if COMP_QT:
                nc.sync.dma_start(
                    qt_lo[:, 0:N_QTC, :, 512:1024],
                    QT_lo_d[:, 0:N_QTC, :, 512:1024],
                )

            # vb before the stage-2 operands (needed at the first exp)
            vb_t = [const.tile([128, 8], f32, name=f"vb_t{p}") for p in range(2)]
            for pp in range(2):
                nc.sync.dma_start(
                    vb_t[pp][:], vb_d[pp].rearrange("(t p) o -> p (t o)", p=128)
                )

            st_hi = [None, None]
            st_lo = [None, None]
            sn_hi = [None, None]
            sn_lo = [None, None]
            ht_hi = [None, None]
            ht_lo = [None, None]

            def load_pairs(pool, tag, name, dram):
                t = pool.tile([128, 4, 2, D], f8, tag=tag, name=name)
                nc.sync.dma_start(t[:], dram[:])
                return t

            # stage-2/3 operands for path 0 follow behind the stage-1 set
            st_hi[0] = load_pairs(stp, "sthi", "st_hi0", ST_hi_d[0])
            st_lo[0] = load_pairs(stp, "stlo", "st_lo0", ST_lo_d[0])

            # remaining small constants (rowsum / sbc / S4)
            ones_t = const.tile([128, 2, 16], f8)
            nc.sync.dma_start(ones_t[:], ones_d[:])
            wgt_sb = const.tile([1, 2], f32)
            nc.sync.dma_start(wgt_sb[:], wgt[:])
            boe_t = const.tile([128, 8], f32)
            nc.sync.dma_start(boe_t[:], boe_c.rearrange("(t p) o -> p (t o)", p=128))

            sn_hi[0] = load_pairs(snp, "snhi", "sn_hi0", SN_hi_d[0])
            sn_lo[0] = load_pairs(snp, "snlo", "sn_lo0", SN_lo_d[0])

            aos_hi = [None, None]
            aos_lo = [None, None]
            sbc = [[None, None], [None, None]]

            for p in range(2):
                if p == 1:
                    # path-1 operands (slots freed by path 0 reuse via tags)
                    g_pair[1] = gp.tile(
                        [128, 4, 2, 2 * D], f8, tag="gpair", name="g_pair1"
                    )
                    nc.sync.dma_start(g_pair[1][:], G_pair_d[1][:])
                    st_hi[1] = load_pairs(stp, "sthi", "st_hi1", ST_hi_d[1])
                    st_lo[1] = load_pairs(stp, "stlo", "st_lo1", ST_lo_d[1])
                    sn_hi[1] = load_pairs(snp, "snhi", "sn_hi1", SN_hi_d[1])
                    sn_lo[1] = load_pairs(snp, "snlo", "sn_lo1", SN_lo_d[1])

                # =====================================================
                # S1: TMP[d',q] = sum_d G[d,d'] QT[d,q]   (DR pairs over d)
                # =====================================================
                tmp_hi = [
                    tmpp.tile([128, 4, 2, 512], f8, tag=f"tmphi{qb}", name="tmp_hi")
                    for qb in range(2)
                ]
                tmp_lo = [
                    tmpp.tile([128, 4, 2, 512], f8, tag=f"tmplo{qb}", name="tmp_lo")
                    for qb in range(2)
                ]
                # (j, term) consumption order matches the DMA delivery order:
                # pair j's hi+lo G terms run back-to-back, QT_lo term last
                s1_sched = [(j, t) for j in range(4) for t in range(2)]
                if COMP_QT:
                    # compensate the QT quantization on the first N_QTC of 4
                    # contraction pairs: the uncompensated remainder is damped
                    # through the softmax (~0.9% final at N_QTC=2, measured on
                    # both observed input sets), inside the 2e-2 budget
                    s1_sched += [(j, 2) for j in range(N_QTC)]
                s1_ops = [(0, qt_hi), (D, qt_hi), (0, qt_lo)]
                n_mm = len(s1_sched)
                def s1_cast(ps, dp, qb):
                    dst_hi = tmp_hi[qb][:, dp // 2, dp % 2, :]
                    nc.scalar.activation(
                        dst_hi, ps[:], Identity, scale=float(ST__ / (SG * SQ))
                    )
                    if COMP_TMP:
                        nc.vector.scalar_tensor_tensor(
                            tmp_lo[qb][:, dp // 2, dp % 2, :],
                            ps[:],
                            float(ST__ / (SG * SQ)),
                            dst_hi,
                            MULT,
                            SUB,
                        )

                if p == 0:
                    # (j, term)-outer in 4-dp halves — consumption tracks DMA
                    # delivery (lo terms last); each half's PSUM banks drain
                    # while the next half runs
                    for q_b0 in range(2):
                        qs = slice(q_b0 * 512, (q_b0 + 1) * 512)
                        for half in range(2):
                            dps = range(half * 4, half * 4 + 4)
                            ps_t = {
                                dp: psp.tile(
                                    [128, 512], f32, tag="acc", name="ps1"
                                )
                                for dp in dps
                            }
                            for k, (j, t) in enumerate(s1_sched):
                                off, qt = s1_ops[t]
                                for dp in dps:
                                    nc.tensor.matmul(
                                        ps_t[dp][:],
                                        g_pair[p][
                                            :, j, :,
                                            off + dp * 128 : off + (dp + 1) * 128,
                                        ],
                                        qt[:, j, :, qs],
                                        start=(k == 0),
                                        stop=(k == n_mm - 1),
                                        perf_mode=DR,
                                    )
                            for dp in dps:
                                s1_cast(ps_t[dp], dp, q_b0)
                    qb1_list = []
                else:
                    qb1_list = [0, 1]

                # dp-outer — staggers PSUM bank release for pipelining
                for q_b1 in qb1_list:
                  qs = slice(q_b1 * 512, (q_b1 + 1) * 512)
                  for dp in range(8):
                    ps = psp.tile([128, 512], f32, tag="acc", name="ps1b")
                    for k, (j, t) in enumerate(s1_sched):
                        off, qt = s1_ops[t]
                        nc.tensor.matmul(
                            ps[:],
                            g_pair[p][
                                :, j, :, off + dp * 128 : off + (dp + 1) * 128
                            ],
                            qt[:, j, :, qs],
                            start=(k == 0),
                            stop=(k == n_mm - 1),
                            perf_mode=DR,
                        )
                    s1_cast(ps, dp, q_b1)
                del qb1_list

                # =====================================================
                # S2: logitsT[k,q] = sum_d' ST[d',k] TMP[d',q] ; exp + hi/lo
                # =====================================================
                exp_hi = [
                    expp.tile([128, 4, 2, 512], f8, tag=f"exphi{qb}", name="exp_hi")
                    for qb in range(2)
                ]
                exp_lo = [
                    expp.tile([128, 4, 2, 512], f8, tag=f"explo{qb}", name="exp_lo")
                    for qb in range(2)
                ]
                s2_terms = [(st_hi[p], tmp_hi), (st_lo[p], tmp_hi)]
                if COMP_TMP:
                    s2_terms.append((st_hi[p], tmp_lo))
                for q_b in range(2):
                    for k_t in range(8):
                        ps = psp.tile([128, 512], f32, tag="acc", name="ps2")
                        n_mm = len(s2_terms) * 4
                        k = 0
                        for (st_, tm) in s2_terms:
                            for j in range(4):
                                nc.tensor.matmul(
                                    ps[:],
                                    st_[:, j, :, k_t * 128 : (k_t + 1) * 128],
                                    tm[q_b][:, j, :, :],
                                    start=(k == 0),
                                    stop=(k == n_mm - 1),
                                    perf_mode=DR,
                                )
                                k += 1
                        scr = scrp.tile([128, 512], f32, tag="scr", name="scr")
                        nc.scalar.activation(
                            scr[:], ps[:], Exp,
                            bias=vb_t[p][:, k_t : k_t + 1],
                            scale=float(SCALE / (ST_ * ST__)),
                        )
                        dst_hi = exp_hi[q_b][:, k_t // 2, k_t % 2, :]
                        nc.scalar.activation(dst_hi, scr[:], Identity)
                        lo_eng = nc.gpsimd if k_t % 2 == 0 else nc.vector
                        lo_eng.tensor_tensor(
                            exp_lo[q_b][:, k_t // 2, k_t % 2, :], scr[:], dst_hi, SUB
                        )

                # rowsum over quantized exp (hi+lo) via DR ones-matmul,
                # then sbc = w * SA/SSN / rowsum broadcast over partitions.
                # Emission is deferred behind covering PE work so the ones-
                # matmuls never stall on the Pool exp_lo chain.
                def emit_rowsum(q_b):
                    # sum of exp_hi only: the exp_lo column-sum is a zero-mean
                    # ~0.06% correction, far below the error budget
                    ps_s = psp.tile([1, 512], f32, tag="acc", name="ps_s")
                    for j in range(4):
                        nc.tensor.matmul(
                            ps_s[:],
                            ones_t[:, :, 0:1],
                            exp_hi[q_b][:, j, :, :],
                            start=(j == 0),
                            stop=(j == 3),
                            perf_mode=DR,
                        )
                    rs = vecp.tile([1, 512], f32, tag="rs", name="rs")
                    nc.vector.reciprocal(rs[:], ps_s[:])
                    s_row = vecp.tile([1, 512], f32, tag="srow", name="s_row")
                    nc.vector.tensor_scalar_mul(
                        s_row[:], rs[:], wgt_sb[0:1, p : p + 1]
                    )
                    srow_d = dramp.tile([1, 512], f32, tag="srd", name="srow_d")
                    nc.sync.dma_start(srow_d[:], s_row[:])
                    sb_t = sbcp.tile([128, 512], f32, tag=f"sbc{q_b}", name="sb_t")
                    nc.sync.dma_start(sb_t[:], srow_d[0:1, :].partition_broadcast(128))
                    sbc[p][q_b] = sb_t

                emit_rowsum(0)

                if p == 0:
                    # path-1 HT prefetch slot behind path-0 compute
                    ht_hi[0] = load_pairs(htp, "hthi0", "ht_hi0", HT_hi_d[0])
                    ht_lo[0] = load_pairs(htp, "htlo0", "ht_lo0", HT_lo_d[0])

                # =====================================================
                # S3: AOS[d',q] = sum_k SN[k,d'] expT[k,q] ; scale + hi/lo
                # =====================================================
                aos_hi[p] = [
                    aosp.tile([128, 4, 2, 512], f8, tag=f"aoshi{p}{qb}",
                              name=f"aos_hi{p}")
                    for qb in range(2)
                ]
                aos_lo[p] = [
                    aosp.tile([128, 4, 2, 512], f8, tag=f"aoslo{p}{qb}",
                              name=f"aos_lo{p}")
                    for qb in range(2)
                ]
                s3_terms = [(sn_hi[p], exp_hi), (sn_lo[p], exp_hi), (sn_hi[p], exp_lo)]

                def emit_s3(q_b, dps):
                    for dp in dps:
                        ps = psp.tile([128, 512], f32, tag="acc", name="ps3")
                        k = 0
                        for (sn, et) in s3_terms:
                            for j in range(4):
                                nc.tensor.matmul(
                                    ps[:],
                                    sn[:, j, :, dp * 128 : (dp + 1) * 128],
                                    et[q_b][:, j, :, :],
                                    start=(k == 0),
                                    stop=(k == 11),
                                    perf_mode=DR,
                                )
                                k += 1
                        t32 = scrp.tile([128, 512], f32, tag="scr", name="t32")
                        nc.vector.tensor_tensor(t32[:], ps[:], sbc[p][q_b][:], MULT)
                        dst_hi = aos_hi[p][q_b][:, dp // 2, dp % 2, :]
                        nc.vector.tensor_copy(dst_hi, t32[:])
                        lo_eng = nc.gpsimd if dp % 2 == 0 else nc.vector
                        lo_eng.tensor_tensor(
                            aos_lo[p][q_b][:, dp // 2, dp % 2, :], t32[:], dst_hi, SUB
                        )

                emit_s3(0, range(0, 4))
                emit_rowsum(1)
                emit_s3(0, range(4, 8))
                emit_s3(1, range(0, 8))

                if p == 0:
                    ht_hi[1] = load_pairs(htp, "hthi1", "ht_hi1", HT_hi_d[1])
                    ht_lo[1] = load_pairs(htp, "htlo1", "ht_lo1", HT_lo_d[1])

            # =====================================================
            # S4: outT[o,q] = sum_p sum_d' HT_p[d',o] AOSs_p[d',q]
            # single accumulation over both paths (contraction 2048 x 3 terms)
            # =====================================================
            s4_terms = [
                (p, ht, at)
                for p in range(2)
                for (ht, at) in (
                    (ht_hi[p], aos_hi[p]),
                    (ht_lo[p], aos_hi[p]),
                    (ht_hi[p], aos_lo[p]),
                )
            ]

            def emit_s4_chain(o_t, q0, width):
                ps = psp.tile([128, width], f32, tag="acc", name="ps4")
                q_b = q0 // 512
                qsl = slice(q0 - q_b * 512, q0 - q_b * 512 + width)
                k = 0
                for (_, ht, at) in s4_terms:
                    for j in range(4):
                        nc.tensor.matmul(
                            ps[:],
                            ht[:, j, :, o_t * 128 : (o_t + 1) * 128],
                            at[q_b][:, j, :, qsl],
                            start=(k == 0),
                            stop=(k == 23),
                            perf_mode=DR,
                        )
                        k += 1
                osb = osbp.tile([128, width], f32, tag="osb", name="osb")
                nc.scalar.activation(
                    osb[:], ps[:], Identity,
                    bias=boe_t[:, o_t : o_t + 1],
                    scale=float(1.0 / (SH * SA)),
                )
                nc.sync.dma_start(
                    outT[o_t * 128 : (o_t + 1) * 128, q0 : q0 + width], osb[:]
                )

            for q_b in range(2):
                for o_t in range(8):
                    if q_b == 1 and o_t == 7:
                        # split the last tile so the final copy+DMA chain is
                        # short and pipelines behind the preceding matmuls
                        emit_s4_chain(o_t, 512, 256)
                        emit_s4_chain(o_t, 768, 128)
                        emit_s4_chain(o_t, 896, 128)
                    else:
                        emit_s4_chain(o_t, q_b * 512, 512)

    nc.compile()
    return nc


def _get_program():
    if "nc" not in _CACHE:
        _CACHE["nc"] = _build_program()
    return _CACHE["nc"]


def _host_gating(Q, Wq, bq, Wm1, bm1, Wm2, bm2):
    """Replicates the reference path-score MLP + top-k sparse weights."""
    Qm = Q.astype(np.float64).mean(axis=1)  # [B, D]
    pooled = Qm @ Wq.astype(np.float64).T + bq.astype(np.float64)
    h = np.maximum(pooled @ Wm1.astype(np.float64).T + bm1.astype(np.float64), 0.0)
    pl = h @ Wm2.astype(np.float64).T + bm2.astype(np.float64)  # [B, P]
    pl = pl - pl.max(axis=1, keepdims=True)
    e = np.exp(pl)
    scores = e / e.sum(axis=1, keepdims=True)
    idx = np.argsort(-scores, axis=1, kind="stable")[:, :TOP_K]  # [B, 2]
    w = np.take_along_axis(scores, idx, axis=1)
    wn = w / (w.sum(axis=1, keepdims=True) + 1e-8)
    return idx.astype(np.int64), wn.astype(np.float32)


def _q8_pair(x, scale):
    """x [1024, C] -> (hi, lo) fp8 pair tensors [128, 4, 2, C].

    Clips to the e4m3 max-normal (+-240) so extreme outliers saturate
    instead of becoming fp8 inf."""
    xs = (np.asarray(x, np.float32) * np.float32(scale)).astype(np.float32)
    xs = np.clip(xs, -240.0, 240.0)
    hi = xs.astype(E4)
    lo = np.clip(xs - hi.astype(np.float32), -240.0, 240.0).astype(E4)
    C = x.shape[1]

    def lay(a):
        return np.ascontiguousarray(
            a.reshape(4, 2, 128, C).transpose(2, 0, 1, 3)
        )

    return lay(hi), lay(lo)


def kernel(**inputs):
    from concourse.bass_utils import run_bass_kernel_spmd

    Q = np.asarray(inputs["Q"], dtype=np.float32)
    src = np.asarray(inputs["src"], dtype=np.float32)
    Wq = np.asarray(inputs["Wq"], dtype=np.float32)
    bq = np.asarray(inputs["bq"], dtype=np.float32)
    Wk = np.asarray(inputs["Wk"], dtype=np.float32)
    Wv = np.asarray(inputs["Wv"], dtype=np.float32)
    bv = np.asarray(inputs["bv"], dtype=np.float32)
    Wm1 = np.asarray(inputs["Wm1"], dtype=np.float32)
    bm1 = np.asarray(inputs["bm1"], dtype=np.float32)
    Wm2 = np.asarray(inputs["Wm2"], dtype=np.float32)
    bm2 = np.asarray(inputs["bm2"], dtype=np.float32)
    Wo = np.asarray(inputs["Wo"], dtype=np.float32)
    bo = np.asarray(inputs["bo"], dtype=np.float32)

    idx, wn = _host_gating(Q, Wq, bq, Wm1, bm1, Wm2, bm2)
    SCALE = 1.0 / float(np.sqrt(D))

    nc = _get_program()

    # host-folded weights, shared across cores (<=4 selected paths)
    sel = sorted(set(idx.flatten().tolist()))
    WqT = Wq.T
    G8 = {
        p: np.ascontiguousarray(np.concatenate(_q8_pair(WqT @ Wk[p], SG), axis=3))
        for p in sel
    }
    HT8 = {p: _q8_pair((Wo @ Wv[p]).T, SH) for p in sel}
    g2 = {p: Wk[p].T @ bq for p in sel}
    Wobv = {p: Wo @ bv[p] for p in sel}
    ones_pair = np.ones((128, 2, 16), dtype=E4)
    LN_SE = float(np.log(SE))

    in_maps = []
    for b in range(B):
        p0, p1 = int(idx[b, 0]), int(idx[b, 1])
        boe = bo + wn[b, 0] * Wobv[p0] + wn[b, 1] * Wobv[p1]
        qt_hi, qt_lo = _q8_pair(Q[b].T, SQ)
        m = {
            "QT_hi": qt_hi,
            "QT_lo": qt_lo,
            "boe": np.ascontiguousarray(boe.reshape(D, 1).astype(np.float32)),
            "wgt": np.ascontiguousarray(
                (wn[b] * (SA / SSN)).reshape(1, 2).astype(np.float32)
            ),
            "ones_pair": ones_pair,
        }
        for i, p in enumerate((p0, p1)):
            S = src[p, b]
            m[f"G{i}_pair"] = G8[p]
            m[f"ST{i}_hi"], m[f"ST{i}_lo"] = _q8_pair(S.T, ST_)
            m[f"SN{i}_hi"], m[f"SN{i}_lo"] = _q8_pair(S, SSN)
            m[f"HT{i}_hi"], m[f"HT{i}_lo"] = HT8[p]
            m[f"vb{i}"] = np.ascontiguousarray(
                ((S @ g2[p]) * SCALE + LN_SE).reshape(LK, 1).astype(np.float32)
            )
        in_maps.append(m)

    res = run_bass_kernel_spmd(nc, in_maps, core_ids=list(range(N_CORES)))
    out = np.stack([res.results[b]["outT"].T for b in range(B)], axis=0)
    return np.ascontiguousarray(out).astype(np.float32)
